# revision 1
# baseline (speedup 1.0000x reference)
"""Block-sparse 3-layer MLP on 8 Trainium2 NeuronCores.

Reference computation (fp32):
    h1 = relu(x @ (W1*expand(mask1)).T + b1)       x:[B,2048] W1:[4096,2048]
    h2 = relu(h1 @ (W2*expand(mask2)).T + b2)      W2:[4096,4096]
    out = h2 @ Wo.T + bo                           Wo:[1024,4096] -> [B,1024]

Strategy: data-parallel over the batch (B=8192 -> 1024 rows/core), no
collectives. Masks are applied to the weights on the host (free) and all
matmuls run dense on the PE array: at density 0.5 with 32x32 mask tiles,
skipping zero tiles via PE-array tiling is slower than dense (packed 32x32
tiles reach only ~36% of dense PE throughput), and fp8 DoubleRow (2x PE
rate) fails the 2e-2 error gate (e4m3 one-pass ~6% rel err; an accurate
3-term hi/lo split needs 1.5x the products, i.e. slower than bf16).

All matmul operands are bf16: same 1 cycle/row PE rate as f32r but half
the DMA bytes and SBUF footprint (rel err ~4e-3, budget 2e-2).
Activations are feature-major [features, batch] so biases are
per-partition and lhsT panels are host-pretransposed [128, K] blocks.

Per core, three phases, PE-saturated throughout:
  L1: 32 m-tiles, psum-accumulated over 16 k-tiles, RELU -> h1 resident
      in SBUF (bf16, 8MB). Inputs stream on the three DMA-capable rings
      (sync/gpsimd/scalar) one descriptor per k-tile in PE consumption
      order, critical tiles at the ring heads.
  L2: 32 m-tiles over 32 k-tiles, RELU -> h2 resident (8MB). ps1+ps2
      (4+4 psum banks) stay open across the L1->L2 boundary: closing a
      psum pool there fences the PE on the layer's last RELU.
  L3: output accumulated directly in PSUM: 2 groups of 4 output m-tiles,
      each group's 4 accumulators [128,1024] = 8 psum banks, k-outer over
      the 32 h2 tiles. No SBUF accumulator and no vector adds; the tail
      is the last bias-activation (split scalar/vector) + output DMA on
      the two HW-DGE rings (gpsimd SW-DGE would serialize the drain).

Scheduling rules learned from traces: DMA-completion semaphores are
coalesced per ring segment, so loads are emitted interleaved with their
consumers and anything first-matmul-critical rides the sync/scalar HW
rings; every tile gets exactly one DMA descriptor (partial writes to a
shared tile serialize rings via write-order semaphores).
"""

import sys

sys.path.insert(0, "/opt/trn_rl_repo")

import numpy as np

from concourse import bacc, mybir, tile
from concourse.bass_utils import run_bass_kernel_spmd

F32 = mybir.dt.float32
BF16 = mybir.dt.bfloat16
RELU = mybir.ActivationFunctionType.Relu
IDENT = mybir.ActivationFunctionType.Identity

N_CORES = 8
TILE = 32  # block-sparse tile size of the masks
P = 128  # partitions


def _build(nc, d_in, d_h, d_out, bc):
    """Emit the per-core kernel. bc = batch columns per core."""
    kt1 = d_in // P  # k-tiles in layer 1 (16)
    mt1 = d_h // P  # m-tiles of h1 == k-tiles of layer 2 (32)
    mt2 = d_h // P  # m-tiles of h2 == k-tiles of layer 3 (32)
    mot = d_out // P  # m-tiles of out (8)
    sw = min(512, bc)  # psum strip width (PE moving-operand max; 1024 crashes walrus)
    ns = bc // sw  # strips per row of tiles
    GW = 4  # output m-tiles per L3 psum group (4 x 2 banks = 8 banks)

    xt_d = nc.dram_tensor("xt", [kt1, P, bc], BF16, kind="ExternalInput")
    w1_d = nc.dram_tensor("w1", [mt1, P, d_in], BF16, kind="ExternalInput")
    b1_d = nc.dram_tensor("b1", [P, mt1], F32, kind="ExternalInput")
    w2_d = nc.dram_tensor("w2", [mt2, P, d_h], BF16, kind="ExternalInput")
    b2_d = nc.dram_tensor("b2", [P, mt2], F32, kind="ExternalInput")
    wo_d = nc.dram_tensor("wo", [mt2, P, d_out], BF16, kind="ExternalInput")
    bo_d = nc.dram_tensor("bo", [P, mot], F32, kind="ExternalInput")
    # output stored bf16: halves the final HBM drain; adds ~0.1% rms to a
    # 0.39% error against a 2% budget
    out_d = nc.dram_tensor("out", [mot, P, bc], BF16, kind="ExternalOutput")

    with tile.TileContext(nc) as tc:
        with (
            tc.tile_pool(name="bias", bufs=1) as bias_pool,
            tc.tile_pool(name="h1", bufs=1) as h1_pool,
            tc.tile_pool(name="h2", bufs=1) as h2_pool,
            tc.tile_pool(name="w2p", bufs=3) as w2_pool,
        ):
            b1_sb = bias_pool.tile([P, mt1], F32, tag="b1")
            b2_sb = bias_pool.tile([P, mt2], F32, tag="b2")
            bo_sb = bias_pool.tile([P, mot], F32, tag="bo")

            h1 = []
            h2 = []
            wo_pre = {}
            if True:
                # ---------------- Layer 1 ----------------
                # ps1 and ps2 stay open across the L1->L2 boundary: closing
                # a psum pool at the boundary fences on the layer's LAST
                # RELU (the pool's final reader), stalling the PE ~1.2us.
                # Only the SBUF pools (last readers: the matmuls themselves)
                # close at the boundary. 4+4 banks; both close before ps3.
                es_ps1 = tc.tile_pool(name="ps1", bufs=2, space="PSUM")
                ps1_pool = es_ps1.__enter__()
                es_ps2 = tc.tile_pool(name="ps2", bufs=2, space="PSUM")
                ps2_pool = es_ps2.__enter__()
                with (
                    tc.tile_pool(name="xtp", bufs=1) as xt_pool,
                    tc.tile_pool(name="w1p", bufs=4) as w1_pool,
                ):
                    # The tile scheduler batches DMA-completion semaphores
                    # per ring segment between consumers, so loads must be
                    # EMITTED interleaved with the matmuls that consume them
                    # or the first matmul waits on a whole batch. Each xt
                    # k-tile is its own SBUF tile written by one descriptor
                    # (slices of a shared tile create cross-ring
                    # write-ordering semaphores); loads pace 3 k-tiles ahead
                    # of consumption, W1 panels 2 ahead, biases on the
                    # scalar ring where slack allows.
                    rings = [nc.sync, nc.gpsimd, nc.scalar]
                    xt = [
                        xt_pool.tile([P, bc], BF16, name=f"xt_{kt}", tag=f"xt_{kt}")
                        for kt in range(kt1)
                    ]

                    def load_xt(kt, eng):
                        eng.dma_start(out=xt[kt][:], in_=xt_d[kt])

                    # critical head on the two HW-DGE rings (sync, scalar);
                    # gpsimd's SW-DGE completion signals are coalesced and
                    # release late, so it only carries loads with slack
                    load_xt(0, nc.scalar)
                    w1pre = {}
                    w2pre = []
                    w1t0 = w1_pool.tile([P, d_in], BF16, tag="w1t")
                    nc.sync.dma_start(out=w1t0[:], in_=w1_d[0])
                    w1pre[0] = w1t0
                    load_xt(1, nc.sync)
                    nc.scalar.dma_start(out=b1_sb[:], in_=b1_d[:])
                    load_xt(2, nc.scalar)
                    load_xt(3, nc.sync)
                    xt_next = 4
                    w1_next = 1

                    def issue_w1(mt):
                        t = w1_pool.tile([P, d_in], BF16, tag="w1t")
                        eng = nc.sync if mt == 1 else rings[mt % 3]
                        eng.dma_start(out=t[:], in_=w1_d[mt])
                        w1pre[mt] = t

                    for mt in range(mt1):
                        while mt > 0 and w1_next <= min(mt + 2, mt1 - 1):
                            issue_w1(w1_next)
                            w1_next += 1
                        w1t = w1pre.pop(mt)
                        ps = ps1_pool.tile([P, bc], F32, tag="ps1")
                        for kt in range(kt1):
                            if mt == 0:
                                while xt_next <= min(kt + 4, kt1 - 1):
                                    if xt_next < 8:
                                        eng = nc.sync if xt_next % 2 else nc.scalar
                                    else:
                                        eng = rings[xt_next % 3]
                                    load_xt(xt_next, eng)
                                    xt_next += 1
                                if kt == 5:
                                    # W1 panels 1,2 once xt's head is in
                                    while w1_next <= 2:
                                        issue_w1(w1_next)
                                        w1_next += 1
                            for n in range(ns):
                                nc.tensor.matmul(
                                    ps[:, n * sw : (n + 1) * sw],
                                    w1t[:, kt * P : (kt + 1) * P],
                                    xt[kt][:, n * sw : (n + 1) * sw],
                                    start=(kt == 0),
                                    stop=(kt == kt1 - 1),
                                )
                        if mt == 0:
                            nc.scalar.dma_start(out=b2_sb[:], in_=b2_d[:])
                            nc.scalar.dma_start(out=bo_sb[:], in_=bo_d[:])
                        if mt == 3:
                            # W2 panels 0,1 on the now-idle scalar ring
                            for i in range(2):
                                t = w2_pool.tile([P, d_h], BF16, tag="w2t")
                                nc.scalar.dma_start(out=t[:], in_=w2_d[i])
                                w2pre.append(t)
                        h = h1_pool.tile(
                            [P, bc], BF16, name=f"h1_{mt}", tag=f"h1_{mt}"
                        )
                        nc.scalar.activation(
                            h[:], ps[:], RELU, bias=b1_sb[:, mt : mt + 1]
                        )
                        h1.append(h)

                # ---------------- Layer 2 ----------------
                es_wop = tc.tile_pool(name="wop", bufs=10)
                wo_pool = es_wop.__enter__()
                w2map = {0: w2pre[0], 1: w2pre[1]}
                w2_next = 2

                def issue_w2(mt):
                    t = w2_pool.tile([P, d_h], BF16, tag="w2t")
                    eng = nc.sync if mt % 2 else nc.gpsimd
                    eng.dma_start(out=t[:], in_=w2_d[mt])
                    w2map[mt] = t

                for mt in range(mt2):
                    while w2_next <= min(mt + 1, mt2 - 1):
                        issue_w2(w2_next)
                        w2_next += 1
                    w2t = w2map.pop(mt)
                    ps = ps2_pool.tile([P, bc], F32, tag="ps2")
                    for kt in range(mt1):
                        for n in range(ns):
                            nc.tensor.matmul(
                                ps[:, n * sw : (n + 1) * sw],
                                w2t[:, kt * P : (kt + 1) * P],
                                h1[kt][:, n * sw : (n + 1) * sw],
                                start=(kt == 0),
                                stop=(kt == mt1 - 1),
                            )
                    h = h2_pool.tile([P, bc], BF16, name=f"h2_{mt}", tag=f"h2_{mt}")
                    if mt == mt2 - 1:
                        # ps2's pool close fences L3 on this RELU; split it
                        # scalar/vector so the fence halves
                        hb = bc // 2
                        nc.scalar.activation(
                            h[:, 0:hb], ps[:, 0:hb], RELU, bias=b2_sb[:, mt : mt + 1]
                        )
                        nc.vector.tensor_scalar(
                            h[:, hb:],
                            ps[:, hb:],
                            b2_sb[:, mt : mt + 1],
                            0.0,
                            mybir.AluOpType.add,
                            mybir.AluOpType.max,
                        )
                    else:
                        nc.scalar.activation(
                            h[:], ps[:], RELU, bias=b2_sb[:, mt : mt + 1]
                        )
                    h2.append(h)
                    if mt == 24:
                        # L3's first half-panels of Wo on the mostly-idle
                        # scalar ring, well ahead of L3's start
                        for kt in range(4):
                            t = wo_pool.tile([P, GW * P], BF16, tag="wot")
                            nc.scalar.dma_start(out=t[:], in_=wo_d[kt][:, 0 : GW * P])
                            wo_pre[kt] = t

            es_ps2.__exit__(None, None, None)
            es_ps1.__exit__(None, None, None)

            # ---------------- Layer 3 ----------------
            # Output accumulates in PSUM across all 32 k-tiles: groups of
            # output m-tiles with one [128, bc] f32 accumulator each (2
            # banks), k-outer; Wo panel slices stream per group. Group
            # sizes [4, 4]: measured faster than [4, 3, 1] -- extra group
            # boundaries (psum-buffer reuse waits on the prior finalize)
            # cost more than the smaller final drain saves.
            GRPS = [GW] * (mot // GW)
            goff = [sum(GRPS[:i]) for i in range(len(GRPS))]
            with (
                tc.tile_pool(name="ps3", bufs=1, space="PSUM") as ps3_pool,
                tc.tile_pool(name="osb", bufs=GW) as osb_pool,
            ):
                seq = [(g, kt) for g in range(len(GRPS)) for kt in range(mt2)]
                wot_map = {(0, kt): t for kt, t in wo_pre.items()}
                wo_next = 0

                def issue_wo(g, kt):
                    if (g, kt) in wot_map:
                        return
                    gw = GRPS[g]
                    t = wo_pool.tile([P, gw * P], BF16, tag="wot")
                    eng = nc.sync if kt % 2 else nc.gpsimd
                    eng.dma_start(
                        out=t[:],
                        in_=wo_d[kt][:, goff[g] * P : (goff[g] + gw) * P],
                    )
                    wot_map[(g, kt)] = t

                for idx, (g, kt) in enumerate(seq):
                    gw = GRPS[g]
                    if kt == 0:
                        pss = [
                            ps3_pool.tile(
                                [P, bc], F32, name=f"ps3_{j}", tag=f"ps3_{j}"
                            )
                            for j in range(gw)
                        ]
                    while wo_next <= min(idx + 4, len(seq) - 1):
                        issue_wo(*seq[wo_next])
                        wo_next += 1
                    wot = wot_map.pop((g, kt))
                    for j in range(gw):
                        for n in range(ns):
                            nc.tensor.matmul(
                                pss[j][:, n * sw : (n + 1) * sw],
                                wot[:, j * P : (j + 1) * P],
                                h2[kt][:, n * sw : (n + 1) * sw],
                                start=(kt == 0),
                                stop=(kt == mt2 - 1),
                            )
                    if kt != mt2 - 1:
                        continue
                    last_g = g == len(GRPS) - 1
                    # finalize all accumulators first (scalar half / vector
                    # half per j), then issue the output DMAs on the two
                    # HW-DGE rings only -- gpsimd SW-DGE copies would hold
                    # the final drain hostage, and DMA issues interleaved on
                    # the scalar queue would delay the activations.
                    osbs = []
                    for j in range(gw):
                        mo = goff[g] + j
                        osb = osb_pool.tile([P, bc], BF16, tag="osb")
                        if j == 0 and not last_g:
                            # single full-width op releases this psum
                            # buffer fastest for the next group
                            nc.scalar.activation(
                                osb[:], pss[j][:], IDENT, bias=bo_sb[:, mo : mo + 1]
                            )
                        else:
                            hb = bc // 2
                            nc.scalar.activation(
                                osb[:, 0:hb],
                                pss[j][:, 0:hb],
                                IDENT,
                                bias=bo_sb[:, mo : mo + 1],
                            )
                            nc.vector.tensor_scalar_add(
                                osb[:, hb:], pss[j][:, hb:], bo_sb[:, mo : mo + 1]
                            )
                        osbs.append(osb)
                    for j in range(gw):
                        mo = goff[g] + j
                        osb = osbs[j]
                        if last_g:
                            # final flush in quarters on both HW rings
                            q_ = bc // 4
                            for qi in range(4):
                                eng = nc.sync if qi % 2 == 0 else nc.scalar
                                eng.dma_start(
                                    out=out_d[mo][:, qi * q_ : (qi + 1) * q_],
                                    in_=osb[:, qi * q_ : (qi + 1) * q_],
                                )
                        else:
                            hb = bc // 2
                            nc.sync.dma_start(
                                out=out_d[mo][:, 0:hb], in_=osb[:, 0:hb]
                            )
                            nc.scalar.dma_start(
                                out=out_d[mo][:, hb:], in_=osb[:, hb:]
                            )

            es_wop.__exit__(None, None, None)

    nc.compile()
    return nc


def _expand_mask(mask, t=TILE):
    return np.repeat(np.repeat(np.asarray(mask, dtype=bool), t, axis=0), t, axis=1)


def _pack_lhsT(w, d_m, d_k):
    """[d_m, d_k] weights -> [d_m/P, P, d_k] panels.

    panel[mt, i, kt*P + j] = w[mt*P + j, kt*P + i], so each [P, P] slice of a
    panel is a ready-to-use lhsT block (partition dim = contraction dim).
    """
    mt, kt = d_m // P, d_k // P
    return np.ascontiguousarray(
        w.reshape(mt, P, kt, P).transpose(0, 3, 2, 1).reshape(mt, P, d_k)
    )


def _pack_out_panels(w, d_m, d_k):
    """[d_m, d_k] weights -> [d_k/P, P, d_m] panels keyed by the k-tile.

    panel[kt, i, mo*P + j] = w[mo*P + j, kt*P + i].
    """
    mt, kt = d_m // P, d_k // P
    return np.ascontiguousarray(
        w.reshape(mt, P, kt, P).transpose(2, 3, 0, 1).reshape(kt, P, d_m)
    )


def _pack_bias(b):
    n = b.shape[0] // P
    return np.ascontiguousarray(b.reshape(n, P).T)


def _run(x, w1e, b1, w2e, b2, wo, bo, d_in, d_h, d_out, n_cores=N_CORES, trace=False):
    b = x.shape[0]
    bc = b // n_cores
    kt1 = d_in // P

    nc = bacc.Bacc("TRN2", target_bir_lowering=False, debug=False, num_devices=n_cores)
    _build(nc, d_in, d_h, d_out, bc)

    np_bf16 = mybir.dt.np(BF16)

    def cvt(a):
        return np.ascontiguousarray(a.astype(np_bf16))

    shared = {
        "w1": cvt(_pack_lhsT(w1e, d_h, d_in)),
        "b1": _pack_bias(b1),
        "w2": cvt(_pack_lhsT(w2e, d_h, d_h)),
        "b2": _pack_bias(b2),
        "wo": cvt(_pack_out_panels(wo, d_out, d_h)),
        "bo": _pack_bias(bo),
    }
    in_maps = []
    for c in range(n_cores):
        xc = x[c * bc : (c + 1) * bc]  # [bc, d_in]
        # xt[kt][p, cc] = xc[cc, kt*128 + p]
        xt = np.ascontiguousarray(xc.T).reshape(kt1, P, bc)
        in_maps.append({"xt": cvt(xt), **shared})

    res = run_bass_kernel_spmd(nc, in_maps, core_ids=list(range(n_cores)), trace=trace)
    outs = []
    for c in range(n_cores):
        outs.append(res.results[c]["out"].reshape(d_out, bc).astype(np.float32))
    full = np.concatenate(outs, axis=1)  # [d_out, B]
    return np.ascontiguousarray(full.T), res


def kernel(x, W1, b1, W2, b2, Wo, bo, mask1, mask2):
    x = np.asarray(x, dtype=np.float32)
    w1e = np.asarray(W1, dtype=np.float32) * _expand_mask(mask1)
    w2e = np.asarray(W2, dtype=np.float32) * _expand_mask(mask2)
    out, _ = _run(
        x,
        w1e,
        np.asarray(b1, np.float32),
        w2e,
        np.asarray(b2, np.float32),
        np.asarray(Wo, np.float32),
        np.asarray(bo, np.float32),
        d_in=2048,
        d_h=4096,
        d_out=1024,
    )
    return out



# revision 6
# speedup vs baseline: 1.0203x; 1.0203x over previous
"""Block-sparse 3-layer MLP on 8 Trainium2 NeuronCores, via 1-level Strassen.

Reference computation (fp32):
    h1 = relu(x @ (W1*expand(mask1)).T + b1)       x:[B,2048] W1:[4096,2048]
    h2 = relu(h1 @ (W2*expand(mask2)).T + b2)      W2:[4096,4096]
    out = h2 @ Wo.T + bo                           Wo:[1024,4096] -> [B,1024]

Strategy: data-parallel over the batch (B=8192 -> bc=1024 per core), no
collectives, feature-major activations [features, batch].  The masks make the
weights block-sparse (32x32 tiles, i.i.d. 0.5 density) but i.i.d. 32-granular
sparsity cannot beat dense on a 128x128 PE (any 4-row/4-col-block packing is
~94% dense by the union bound) and fp8 fails the 2e-2 gate (measured 6.2e-2
one-pass, 4.4e-2 with a 2-term split).  Dense bf16 streams at 216ns per
[128x128]x[128x512] matmul and the dense baseline already ran at 95.4% PE
occupancy, so the remaining lever is cutting PE work itself:

Each layer h = W.x runs 1-level Strassen on the 2x2 block split of W
([m/2,k/2] quadrants) and of the feature-major activation ([k/2, 512]
quadrants; the 1024 batch splits into two 512 halves):
    M1=(A11+A22)(B11+B22) M2=(A21+A22)B11 M3=A11(B12-B22) M4=A22(B21-B11)
    M5=(A11+A12)B22 M6=(A21-A11)(B11+B12) M7=(A12-A22)(B21+B22)
    C11=M1+M4-M5+M7 C12=M3+M5 C21=M2+M4 C22=M1-M2+M3+M6
7 half-size products instead of 8: 3136 matmuls/core vs 3584 dense (PE floor
677us vs 773us).  Measured numeric cost: 7.5e-3 rel err vs 3.9e-3 dense bf16.

- A-side combos are free (host precomputes 7 bf16 lhsT panel sets per layer;
  1.75x weight HBM bytes ~ 103MB/core ~ 150GB/s sustained, well within ring
  fanout).  x-side B combos are host-computed too (x is an input).
- h1's B-combos for L2 are built on-device per row tile (5 bf16
  tensor_tensor adds on gpsimd, overlapped with products); h1's C12/C21
  quadrant tiles are freed right after (rotating pool) since only C11/C22
  are consumed raw (M2/M5).  h2's combos cannot coexist with h1's in SBUF,
  so they are deferred to L3 start and built just-in-time under L3 row 0's
  M2/M5 products (which read raw h2 quadrants), split across gpsimd+vector.
- C-combines run on the vector engine as scalar_tensor_tensor chains
  (out=(in0 op0 scalar) op1 in1), each reading exactly one PSUM operand (ISA
  limit) plus one SBUF f32 partial.  M1/M4/M5 are evicted to SBUF by the
  scalar engine (activation IDENT); bias rides the STT scalar slot; relu is
  a vector tensor_scalar_max into the resident bf16 h tiles.  In L3 the
  final STT of each quadrant writes the bf16 output tile directly.
- PSUM: every product accumulates over k/2 into its own [128,512] f32 tile
  (exactly one PSUM bank); peak ~4 live banks per row, one 8-deep pool
  rotates across rows and layers without PE stalls.
- lhsT panels stream per-product on the three DMA rings (sync/scalar/
  gpsimd), 1-2 products ahead; row-0 panels and the first x tiles are split
  finer so the first matmul issues ~2us in.
"""

import sys

sys.path.insert(0, "/opt/trn_rl_repo")

import numpy as np

from concourse import bacc, mybir, tile
from concourse.bass_utils import run_bass_kernel_spmd

F32 = mybir.dt.float32
BF16 = mybir.dt.bfloat16
IDENT = mybir.ActivationFunctionType.Identity
ADD = mybir.AluOpType.add
SUB = mybir.AluOpType.subtract
MULT = mybir.AluOpType.mult

N_CORES = 8
TILE = 32
P = 128
NH = 512  # half-batch strip = one psum bank

# per-row product order: raw-B products first (startup / lazy-combo cover),
# early-evicted M1 next, then combine-dependency order
PRODUCT_ORDER = [2, 5, 1, 4, 6, 3, 7]
PROD_B = {1: "g1", 2: "b11", 3: "g3", 4: "g4", 5: "b22", 6: "g6", 7: "g7"}


def _build(nc, d_in, d_h, d_out, bc):
    kt1 = d_in // 2 // P   # 8  k-tiles per L1 product
    kt2 = d_h // 2 // P    # 16 k-tiles per L2/L3 product
    rt12 = d_h // 2 // P   # 16 row tiles per L1/L2 quadrant
    rt3 = d_out // 2 // P  # 4  row tiles per L3 quadrant

    xb11_d = nc.dram_tensor("xb11", [kt1, P, NH], BF16, kind="ExternalInput")
    xb22_d = nc.dram_tensor("xb22", [2, P, 4 * NH], BF16, kind="ExternalInput")
    xc_d = {
        j: nc.dram_tensor(f"xc{j}", [P, kt1 * NH], BF16, kind="ExternalInput")
        for j in (1, 3, 4, 6, 7)
    }
    w1_d = {j: nc.dram_tensor(f"w1_{j}", [rt12, P, kt1 * P], BF16,
                              kind="ExternalInput") for j in range(1, 8)}
    w2_d = {j: nc.dram_tensor(f"w2_{j}", [rt12, P, kt2 * P], BF16,
                              kind="ExternalInput") for j in range(1, 8)}
    wo_d = {j: nc.dram_tensor(f"wo_{j}", [rt3, P, kt2 * P], BF16,
                              kind="ExternalInput") for j in range(1, 8)}
    b1t_d = nc.dram_tensor("b1t", [P, rt12], F32, kind="ExternalInput")
    b1b_d = nc.dram_tensor("b1b", [P, rt12], F32, kind="ExternalInput")
    b2t_d = nc.dram_tensor("b2t", [P, rt12], F32, kind="ExternalInput")
    b2b_d = nc.dram_tensor("b2b", [P, rt12], F32, kind="ExternalInput")
    bot_d = nc.dram_tensor("bot", [P, rt3], F32, kind="ExternalInput")
    bob_d = nc.dram_tensor("bob", [P, rt3], F32, kind="ExternalInput")
    out_d = nc.dram_tensor("out", [4 * rt3, P, NH], BF16, kind="ExternalOutput")

    with tile.TileContext(nc) as tc:
        with (
            tc.tile_pool(name="bias", bufs=1) as bias_pool,
            tc.tile_pool(name="ev", bufs=3) as ev_pool,
            tc.tile_pool(name="ch", bufs=6) as ch_pool,
            tc.tile_pool(name="ps", bufs=8, space="PSUM") as ps_pool,
        ):
            b1t = bias_pool.tile([P, rt12], F32, tag="b1t")
            b1b = bias_pool.tile([P, rt12], F32, tag="b1b")
            b2t = bias_pool.tile([P, rt12], F32, tag="b2t")
            b2b = bias_pool.tile([P, rt12], F32, tag="b2b")
            bot = bias_pool.tile([P, rt3], F32, tag="bot")
            bob = bias_pool.tile([P, rt3], F32, tag="bob")
            rings = [nc.sync, nc.scalar, nc.gpsimd]

            def emit_layer(lay, rows, kts, rhs, bt_sb, bb_sb, panel_pool,
                           panel_dram, panel_w, lookahead, prefetch_hook,
                           row_hook, out_cb):
                panels = {}
                pf = {"next": 0}
                order = [(r, jp) for r in range(rows) for jp in range(7)]

                def issue_panel(idx, split):
                    r, jp = order[idx]
                    j = PRODUCT_ORDER[jp]
                    t = panel_pool.tile([P, panel_w], BF16, tag=f"pan{lay}")
                    if split == 1:
                        rings[idx % 3].dma_start(out=t[:], in_=panel_dram[j][r])
                    else:
                        w = panel_w // split
                        for s in range(split):
                            rings[(idx + s) % 3].dma_start(
                                out=t[:, s * w:(s + 1) * w],
                                in_=panel_dram[j][r][:, s * w:(s + 1) * w],
                            )
                    panels[(r, j)] = t

                def pump(upto):
                    while pf["next"] <= min(upto, len(order) - 1):
                        issue_panel(
                            pf["next"],
                            2 if (lay == 1 and pf["next"] < 2) else 1,
                        )
                        pf["next"] += 1

                pump(0)
                for r in range(rows):
                    ps = {}
                    e = {}
                    bt = bt_sb[:, r:r + 1]
                    bb = bb_sb[:, r:r + 1]
                    for jp, j in enumerate(PRODUCT_ORDER):
                        idx = r * 7 + jp
                        pump(idx + lookahead)
                        prefetch_hook(r, jp)
                        pan = panels.pop((r, j))
                        pst = ps_pool.tile([P, NH], F32, tag="ps")
                        for kt in range(kts):
                            nc.tensor.matmul(
                                pst[:],
                                pan[:, kt * P:(kt + 1) * P],
                                rhs(j, kt),
                                start=(kt == 0),
                                stop=(kt == kts - 1),
                            )
                        ps[j] = pst
                        # combine DAG, emitted as operands become available
                        if j == 5:
                            e[5] = ev_pool.tile([P, NH], F32, name="e5", tag="ev")
                            nc.scalar.activation(e[5][:], ps[5][:], IDENT)
                        elif j == 1:
                            e[1] = ev_pool.tile([P, NH], F32, name="e1", tag="ev")
                            nc.scalar.activation(e[1][:], ps[1][:], IDENT)
                        elif j == 4:
                            e[4] = ev_pool.tile([P, NH], F32, name="e4", tag="ev")
                            nc.scalar.activation(e[4][:], ps[4][:], IDENT)
                            # C21 = M2 + M4 + bb
                            s21 = ch_pool.tile([P, NH], F32, tag="ch")
                            nc.vector.scalar_tensor_tensor(
                                s21[:], ps[2][:], bb, e[4][:], ADD, ADD)
                            out_cb("c21", r, s21)
                        elif j == 6:
                            # C22 = M1 - M2 + M3 + M6 + bb
                            d1 = ch_pool.tile([P, NH], F32, tag="ch")
                            nc.vector.scalar_tensor_tensor(
                                d1[:], ps[6][:], bb, e[1][:], ADD, ADD)
                            d2 = ch_pool.tile([P, NH], F32, tag="ch")
                            nc.vector.scalar_tensor_tensor(
                                d2[:], ps[2][:], -1.0, d1[:], MULT, ADD)
                            e["d2"] = d2
                        elif j == 3:
                            # C12 = M3 + M5 + bt
                            s12 = ch_pool.tile([P, NH], F32, tag="ch")
                            nc.vector.scalar_tensor_tensor(
                                s12[:], ps[3][:], bt, e[5][:], ADD, ADD)
                            out_cb("c12", r, s12)
                            d3 = ch_pool.tile([P, NH], F32, tag="ch")
                            nc.vector.scalar_tensor_tensor(
                                d3[:], ps[3][:], 1.0, e["d2"][:], MULT, ADD)
                            out_cb("c22", r, d3)
                        elif j == 7:
                            # C11 = M1 + M4 - M5 + M7 + bt
                            c1 = ch_pool.tile([P, NH], F32, tag="ch")
                            nc.vector.scalar_tensor_tensor(
                                c1[:], ps[7][:], bt, e[1][:], ADD, ADD)
                            c2 = ch_pool.tile([P, NH], F32, tag="ch")
                            nc.vector.scalar_tensor_tensor(
                                c2[:], e[4][:], 1.0, c1[:], MULT, ADD)
                            c3 = ch_pool.tile([P, NH], F32, tag="ch")
                            nc.vector.scalar_tensor_tensor(
                                c3[:], e[5][:], -1.0, c2[:], MULT, ADD)
                            out_cb("c11", r, c3)
                    row_hook(r)

            # =================== Layer 1 ===================
            es_h1c11 = tc.tile_pool(name="h1c11", bufs=1, side="left")
            h1c11_pool = es_h1c11.__enter__()
            es_h1c22 = tc.tile_pool(name="h1c22", bufs=1, side="left")
            h1c22_pool = es_h1c22.__enter__()
            es_g = tc.tile_pool(name="g", bufs=1, side="left")
            g_pool = es_g.__enter__()
            h1c11, h1c22 = [], []
            g = {j: [] for j in (1, 3, 4, 6, 7)}

            es_x = tc.tile_pool(name="x", bufs=1, side="left")
            x_pool = es_x.__enter__()
            es_w1 = tc.tile_pool(name="w1", bufs=4, side="left")
            w1_pool = es_w1.__enter__()
            es_h1rot = tc.tile_pool(name="h1rot", bufs=3, side="left")
            h1rot_pool = es_h1rot.__enter__()

            xb11 = [x_pool.tile([P, NH], BF16, name=f"xb11_{k}", tag=f"xb11_{k}")
                    for k in range(kt1)]
            xb22 = [x_pool.tile([P, 4 * NH], BF16, name=f"xb22_{k}", tag=f"xb22_{k}")
                    for k in range(2)]
            xc = {j: x_pool.tile([P, kt1 * NH], BF16, name=f"xc{j}", tag=f"xc{j}")
                  for j in (1, 3, 4, 6, 7)}
            nc.sync.dma_start(out=xb11[0][:], in_=xb11_d[0])
            nc.scalar.dma_start(out=xb11[1][:], in_=xb11_d[1])
            nc.gpsimd.dma_start(out=xb11[2][:], in_=xb11_d[2])
            xc_order = [1, 4, 6, 3, 7]
            st = {"xc": 0}

            def l1_prefetch(r, jp):
                if r == 0 and jp == 0:
                    for k in range(3, kt1):
                        rings[k % 3].dma_start(out=xb11[k][:], in_=xb11_d[k])
                    nc.sync.dma_start(out=xb22[0][:], in_=xb22_d[0])
                    nc.gpsimd.dma_start(out=xb22[1][:], in_=xb22_d[1])
                    nc.scalar.dma_start(out=b1t[:], in_=b1t_d[:])
                    nc.scalar.dma_start(out=b1b[:], in_=b1b_d[:])
                if r == 0 and jp < 5 and st["xc"] <= jp:
                    j = xc_order[st["xc"]]
                    rings[st["xc"] % 3].dma_start(out=xc[j][:], in_=xc_d[j][:])
                    st["xc"] += 1
                if r == 1 and jp == 0:
                    nc.scalar.dma_start(out=b2t[:], in_=b2t_d[:])
                    nc.scalar.dma_start(out=b2b[:], in_=b2b_d[:])

            def l1_rhs(j, kt):
                b = PROD_B[j]
                if b == "b11":
                    return xb11[kt][:]
                if b == "b22":
                    return xb22[kt // 4][:, (kt % 4) * NH:(kt % 4 + 1) * NH]
                return xc[j][:, kt * NH:(kt + 1) * NH]

            rowstate = {}

            def l1_out(quad, r, src):
                if quad == "c11":
                    t = h1c11_pool.tile([P, NH], BF16, name=f"h1c11_{r}",
                                        tag=f"h1c11_{r}")
                    h1c11.append(t)
                elif quad == "c22":
                    t = h1c22_pool.tile([P, NH], BF16, name=f"h1c22_{r}",
                                        tag=f"h1c22_{r}")
                    h1c22.append(t)
                else:
                    t = h1rot_pool.tile([P, NH], BF16, tag=f"h1rot_{quad}")
                    rowstate[quad] = t
                nc.vector.tensor_scalar_max(t[:], src[:], 0.0)

            def l1_row_hook(r):
                c12, c21 = rowstate["c12"], rowstate["c21"]
                c11, c22 = h1c11[r], h1c22[r]
                for j, (a, b, op) in (
                    (1, (c11, c22, ADD)), (3, (c12, c22, SUB)),
                    (4, (c21, c11, SUB)), (6, (c11, c12, ADD)),
                    (7, (c21, c22, ADD)),
                ):
                    t = g_pool.tile([P, NH], BF16, name=f"g{j}_{r}", tag=f"g{j}_{r}")
                    nc.gpsimd.tensor_tensor(t[:], a[:], b[:], op)
                    g[j].append(t)

            emit_layer(1, rt12, kt1, l1_rhs, b1t, b1b, w1_pool, w1_d,
                       kt1 * P, 2, l1_prefetch, l1_row_hook, l1_out)

            es_h1rot.__exit__(None, None, None)
            es_w1.__exit__(None, None, None)
            es_x.__exit__(None, None, None)

            # =================== Layer 2 ===================
            es_h2c11 = tc.tile_pool(name="h2c11", bufs=1, side="right")
            h2c11_pool = es_h2c11.__enter__()
            es_h2c22 = tc.tile_pool(name="h2c22", bufs=1, side="right")
            h2c22_pool = es_h2c22.__enter__()
            es_h2c12 = tc.tile_pool(name="h2c12", bufs=1, side="right")
            h2c12_pool = es_h2c12.__enter__()
            es_h2c21 = tc.tile_pool(name="h2c21", bufs=1, side="right")
            h2c21_pool = es_h2c21.__enter__()
            es_w2 = tc.tile_pool(name="w2", bufs=3, side="left")
            w2_pool = es_w2.__enter__()
            h2 = {"c11": [], "c12": [], "c21": [], "c22": []}
            h2pools = {"c11": h2c11_pool, "c12": h2c12_pool,
                       "c21": h2c21_pool, "c22": h2c22_pool}

            def l2_rhs(j, kt):
                b = PROD_B[j]
                if b == "b11":
                    return h1c11[kt][:]
                if b == "b22":
                    return h1c22[kt][:]
                return g[j][kt][:]

            def l2_out(quad, r, src):
                t = h2pools[quad].tile([P, NH], BF16, name=f"h2{quad}_{r}",
                                       tag=f"h2{quad}_{r}")
                nc.vector.tensor_scalar_max(t[:], src[:], 0.0)
                h2[quad].append(t)

            emit_layer(2, rt12, kt2, l2_rhs, b2t, b2b, w2_pool, w2_d,
                       kt2 * P, 1, lambda r, jp: None, lambda r: None, l2_out)

            es_w2.__exit__(None, None, None)
            es_g.__exit__(None, None, None)
            es_h1c22.__exit__(None, None, None)
            es_h1c11.__exit__(None, None, None)

            # =================== Layer 3 ===================
            es_d = tc.tile_pool(name="d", bufs=1, side="right")
            d_pool = es_d.__enter__()
            es_wo = tc.tile_pool(name="wo", bufs=4, side="right")
            wo_pool = es_wo.__enter__()
            es_osb = tc.tile_pool(name="osb", bufs=8, side="right")
            osb_pool = es_osb.__enter__()
            d = {j: [] for j in (1, 3, 4, 6, 7)}
            d_specs = []
            for j, (qa, qb, op) in (
                (1, ("c11", "c22", ADD)), (4, ("c21", "c11", SUB)),
                (6, ("c11", "c12", ADD)), (3, ("c12", "c22", SUB)),
                (7, ("c21", "c22", ADD)),
            ):
                for kt in range(kt2):
                    d_specs.append((j, kt, qa, qb, op))
            d_engs = [nc.gpsimd, nc.vector]
            dst = {"next": 0}

            def build_d(n):
                while dst["next"] < min(n, len(d_specs)):
                    i = dst["next"]
                    j, kt, qa, qb, op = d_specs[i]
                    t = d_pool.tile([P, NH], BF16, name=f"d{j}_{kt}",
                                    tag=f"d{j}_{kt}")
                    d_engs[i % 2].tensor_tensor(t[:], h2[qa][kt][:], h2[qb][kt][:], op)
                    d[j].append(t)
                    dst["next"] += 1

            def l3_prefetch(r, jp):
                if r == 0:
                    if jp == 0:
                        nc.scalar.dma_start(out=bot[:], in_=bot_d[:])
                        nc.scalar.dma_start(out=bob[:], in_=bob_d[:])
                        build_d(12)
                    elif jp == 1:
                        build_d(kt2 + 12)
                    else:
                        build_d(jp * kt2 + 12)
                elif r == 1 and jp < 2:
                    build_d(len(d_specs))

            def l3_rhs(j, kt):
                b = PROD_B[j]
                if b == "b11":
                    return h2["c11"][kt][:]
                if b == "b22":
                    return h2["c22"][kt][:]
                return d[j][kt][:]

            oq = {"c11": 0, "c12": 1, "c21": 2, "c22": 3}
            ost = {"n": 0}

            def l3_out(quad, r, src):
                # src is bf16; final STT wrote it directly (bias, no relu)
                mo = oq[quad] * rt3 + r
                if r == rt3 - 1:
                    h = NH // 2
                    nc.sync.dma_start(out=out_d[mo][:, 0:h], in_=src[:, 0:h])
                    nc.scalar.dma_start(out=out_d[mo][:, h:], in_=src[:, h:])
                else:
                    eng = nc.sync if ost["n"] % 2 else nc.scalar
                    eng.dma_start(out=out_d[mo], in_=src[:])
                    ost["n"] += 1

            panels3 = {}
            pf3 = {"next": 0}
            order3 = [(r, jp) for r in range(rt3) for jp in range(7)]

            def pump3(upto):
                while pf3["next"] <= min(upto, len(order3) - 1):
                    idx = pf3["next"]
                    r, jp = order3[idx]
                    j = PRODUCT_ORDER[jp]
                    t = wo_pool.tile([P, kt2 * P], BF16, tag="pan3")
                    rings[idx % 3].dma_start(out=t[:], in_=wo_d[j][r])
                    panels3[(r, j)] = t
                    pf3["next"] += 1

            pump3(0)
            for r in range(rt3):
                ps = {}
                e = {}
                bt = bot[:, r:r + 1]
                bb = bob[:, r:r + 1]
                for jp, j in enumerate(PRODUCT_ORDER):
                    idx = r * 7 + jp
                    pump3(idx + 2)
                    l3_prefetch(r, jp)
                    pan = panels3.pop((r, j))
                    pst = ps_pool.tile([P, NH], F32, tag="ps")
                    for kt in range(kt2):
                        nc.tensor.matmul(
                            pst[:],
                            pan[:, kt * P:(kt + 1) * P],
                            l3_rhs(j, kt),
                            start=(kt == 0),
                            stop=(kt == kt2 - 1),
                        )
                    ps[j] = pst
                    if j == 5:
                        e[5] = ev_pool.tile([P, NH], F32, name="e5", tag="ev")
                        nc.scalar.activation(e[5][:], ps[5][:], IDENT)
                    elif j == 1:
                        e[1] = ev_pool.tile([P, NH], F32, name="e1", tag="ev")
                        nc.scalar.activation(e[1][:], ps[1][:], IDENT)
                    elif j == 4:
                        e[4] = ev_pool.tile([P, NH], F32, name="e4", tag="ev")
                        nc.scalar.activation(e[4][:], ps[4][:], IDENT)
                        o21 = osb_pool.tile([P, NH], BF16, tag="osb")
                        nc.vector.scalar_tensor_tensor(
                            o21[:], ps[2][:], bb, e[4][:], ADD, ADD)
                        l3_out("c21", r, o21)
                    elif j == 6:
                        d1 = ch_pool.tile([P, NH], F32, tag="ch")
                        nc.vector.scalar_tensor_tensor(
                            d1[:], ps[6][:], bb, e[1][:], ADD, ADD)
                        d2 = ch_pool.tile([P, NH], F32, tag="ch")
                        nc.vector.scalar_tensor_tensor(
                            d2[:], ps[2][:], -1.0, d1[:], MULT, ADD)
                        e["d2"] = d2
                    elif j == 3:
                        o12 = osb_pool.tile([P, NH], BF16, tag="osb")
                        nc.vector.scalar_tensor_tensor(
                            o12[:], ps[3][:], bt, e[5][:], ADD, ADD)
                        l3_out("c12", r, o12)
                        o22 = osb_pool.tile([P, NH], BF16, tag="osb")
                        nc.vector.scalar_tensor_tensor(
                            o22[:], ps[3][:], 1.0, e["d2"][:], MULT, ADD)
                        l3_out("c22", r, o22)
                    elif j == 7:
                        c1 = ch_pool.tile([P, NH], F32, tag="ch")
                        nc.vector.scalar_tensor_tensor(
                            c1[:], ps[7][:], bt, e[1][:], ADD, ADD)
                        c2 = ch_pool.tile([P, NH], F32, tag="ch")
                        nc.vector.scalar_tensor_tensor(
                            c2[:], e[4][:], 1.0, c1[:], MULT, ADD)
                        o11 = osb_pool.tile([P, NH], BF16, tag="osb")
                        nc.vector.scalar_tensor_tensor(
                            o11[:], e[5][:], -1.0, c2[:], MULT, ADD)
                        l3_out("c11", r, o11)

            es_osb.__exit__(None, None, None)
            es_wo.__exit__(None, None, None)
            es_d.__exit__(None, None, None)
            es_h2c21.__exit__(None, None, None)
            es_h2c12.__exit__(None, None, None)
            es_h2c22.__exit__(None, None, None)
            es_h2c11.__exit__(None, None, None)

    nc.compile()
    return nc


def _expand_mask(mask, t=TILE):
    return np.repeat(np.repeat(np.asarray(mask, dtype=bool), t, axis=0), t, axis=1)


def _pack_lhsT(w):
    """[d_m, d_k] -> [d_m/P, P, d_k] lhsT panels (partition = contraction)."""
    d_m, d_k = w.shape
    mt, kt = d_m // P, d_k // P
    return np.ascontiguousarray(
        w.reshape(mt, P, kt, P).transpose(0, 3, 2, 1).reshape(mt, P, d_k)
    )


def _strassen_a(w):
    m, k = w.shape
    mh, kh = m // 2, k // 2
    A11, A12 = w[:mh, :kh], w[:mh, kh:]
    A21, A22 = w[mh:, :kh], w[mh:, kh:]
    return {
        1: A11 + A22, 2: A21 + A22, 3: A11, 4: A22,
        5: A11 + A12, 6: A21 - A11, 7: A12 - A22,
    }


def _pack_bias(b):
    n = b.shape[0] // P
    return np.ascontiguousarray(b.reshape(n, P).T)


def _run(x, w1e, b1, w2e, b2, wo, bo, d_in, d_h, d_out, n_cores=N_CORES, trace=False):
    b = x.shape[0]
    bc = b // n_cores
    assert bc == 2 * NH

    nc = bacc.Bacc("TRN2", target_bir_lowering=False, debug=False, num_devices=n_cores)
    _build(nc, d_in, d_h, d_out, bc)

    np_bf16 = mybir.dt.np(BF16)

    def cvt(a):
        return np.ascontiguousarray(a.astype(np_bf16))

    shared = {}
    for name, w in (("w1", w1e), ("w2", w2e), ("wo", wo)):
        for j, a in _strassen_a(w).items():
            shared[f"{name}_{j}"] = cvt(_pack_lhsT(a))
    shared["b1t"] = _pack_bias(b1[:d_h // 2])
    shared["b1b"] = _pack_bias(b1[d_h // 2:])
    shared["b2t"] = _pack_bias(b2[:d_h // 2])
    shared["b2b"] = _pack_bias(b2[d_h // 2:])
    shared["bot"] = _pack_bias(bo[:d_out // 2])
    shared["bob"] = _pack_bias(bo[d_out // 2:])

    kh1 = d_in // 2
    kt1 = kh1 // P
    in_maps = []
    for c in range(n_cores):
        xc_ = np.ascontiguousarray(x[c * bc:(c + 1) * bc].T)  # [d_in, bc]
        B11 = xc_[:kh1, :NH]
        B12 = xc_[:kh1, NH:]
        B21 = xc_[kh1:, :NH]
        B22 = xc_[kh1:, NH:]
        m = {
            "xb11": cvt(B11.reshape(kt1, P, NH)),
            "xb22": cvt(
                B22.reshape(2, 4, P, NH).transpose(0, 2, 1, 3).reshape(2, P, 4 * NH)
            ),
        }
        for j, comb in (
            (1, B11 + B22), (3, B12 - B22), (4, B21 - B11),
            (6, B11 + B12), (7, B21 + B22),
        ):
            m[f"xc{j}"] = cvt(
                comb.reshape(kt1, P, NH).transpose(1, 0, 2).reshape(P, kt1 * NH)
            )
        in_maps.append({**m, **shared})

    res = run_bass_kernel_spmd(nc, in_maps, core_ids=list(range(n_cores)), trace=trace)
    outs = []
    rt3 = d_out // 2 // P
    for c in range(n_cores):
        o = res.results[c]["out"].reshape(4, rt3 * P, NH).astype(np.float32)
        full = np.empty((d_out, bc), np.float32)
        full[:rt3 * P, :NH] = o[0]
        full[:rt3 * P, NH:] = o[1]
        full[rt3 * P:, :NH] = o[2]
        full[rt3 * P:, NH:] = o[3]
        outs.append(full)
    full = np.concatenate(outs, axis=1)  # [d_out, B]
    return np.ascontiguousarray(full.T), res


def kernel(x, W1, b1, W2, b2, Wo, bo, mask1, mask2):
    x = np.asarray(x, dtype=np.float32)
    w1e = np.asarray(W1, dtype=np.float32) * _expand_mask(mask1)
    w2e = np.asarray(W2, dtype=np.float32) * _expand_mask(mask2)
    out, _ = _run(
        x,
        w1e,
        np.asarray(b1, np.float32),
        w2e,
        np.asarray(b2, np.float32),
        np.asarray(Wo, np.float32),
        np.asarray(bo, np.float32),
        d_in=2048,
        d_h=4096,
        d_out=1024,
    )
    return out


# revision 8
# speedup vs baseline: 1.1089x; 1.0869x over previous
"""Block-sparse 3-layer MLP on 8 Trainium2 NeuronCores, via 1-level Strassen.

Reference computation (fp32):
    h1 = relu(x @ (W1*expand(mask1)).T + b1)       x:[B,2048] W1:[4096,2048]
    h2 = relu(h1 @ (W2*expand(mask2)).T + b2)      W2:[4096,4096]
    out = h2 @ Wo.T + bo                           Wo:[1024,4096] -> [B,1024]

Strategy: data-parallel over the batch (B=8192 -> bc=1024 per core), no
collectives, feature-major activations [features, batch].  The masks make the
weights block-sparse (32x32 tiles, i.i.d. 0.5 density) but i.i.d. 32-granular
sparsity cannot beat dense on a 128x128 PE (any 4-row/4-col-block packing is
~94% dense by the union bound) and fp8 fails the 2e-2 gate (measured 6.2e-2
one-pass, 4.4e-2 with a 2-term split).  Dense bf16 streams at 216ns per
[128x128]x[128x512] matmul and the dense baseline already ran at 95.4% PE
occupancy, so the remaining lever is cutting PE work itself:

Each layer h = W.x runs 1-level Strassen on the 2x2 block split of W
([m/2,k/2] quadrants) and of the feature-major activation ([k/2, 512]
quadrants; the 1024 batch splits into two 512 halves):
    M1=(A11+A22)(B11+B22) M2=(A21+A22)B11 M3=A11(B12-B22) M4=A22(B21-B11)
    M5=(A11+A12)B22 M6=(A21-A11)(B11+B12) M7=(A12-A22)(B21+B22)
    C11=M1+M4-M5+M7 C12=M3+M5 C21=M2+M4 C22=M1-M2+M3+M6
7 half-size products instead of 8: 3136 matmuls/core vs 3584 dense (PE floor
677us vs 773us).  Measured numeric cost: 7.5e-3 rel err vs 3.9e-3 dense bf16.

- A-side combos are free (host precomputes 7 bf16 lhsT panel sets per layer;
  1.75x weight HBM bytes ~ 103MB/core ~ 150GB/s sustained, well within ring
  fanout).  x-side B combos are host-computed too (x is an input).
- h1's B-combos for L2 are built on-device per row tile (5 bf16
  tensor_tensor adds on gpsimd, overlapped with products); h1's C12/C21
  quadrant tiles are freed right after (rotating pool) since only C11/C22
  are consumed raw (M2/M5).  h2's combos cannot coexist with h1's in SBUF,
  so they are deferred to L3 start and built just-in-time under L3 row 0's
  M2/M5 products (which read raw h2 quadrants), split across gpsimd+vector.
- C-combines run on the vector engine as scalar_tensor_tensor chains
  (out=(in0 op0 scalar) op1 in1), each reading exactly one PSUM operand (ISA
  limit) plus one SBUF f32 partial.  M1/M4/M5 are evicted to SBUF by the
  scalar engine (activation IDENT); bias rides the STT scalar slot; relu is
  a vector tensor_scalar_max into the resident bf16 h tiles.  In L3 the
  final STT of each quadrant writes the bf16 output tile directly.
- PSUM: every product accumulates over k/2 into its own [128,512] f32 tile
  (exactly one PSUM bank); peak ~4 live banks per row, one 8-deep pool
  rotates across rows and layers without PE stalls.
- lhsT panels stream per-product on the three DMA rings (sync/scalar/
  gpsimd), 1-2 products ahead; row-0 panels and the first x tiles are split
  finer so the first matmul issues ~2us in.
"""

import sys

sys.path.insert(0, "/opt/trn_rl_repo")

import numpy as np

from concourse import bacc, mybir, tile
from concourse.bass_utils import run_bass_kernel_spmd

F32 = mybir.dt.float32
BF16 = mybir.dt.bfloat16
IDENT = mybir.ActivationFunctionType.Identity
ADD = mybir.AluOpType.add
SUB = mybir.AluOpType.subtract
MULT = mybir.AluOpType.mult

N_CORES = 8
TILE = 32
P = 128
NH = 512  # half-batch strip = one psum bank

# per-row product order: raw-B products first (startup / lazy-combo cover),
# early-evicted M1 next, then combine-dependency order
PRODUCT_ORDER = [2, 5, 1, 4, 6, 3, 7]
PROD_B = {1: "g1", 2: "b11", 3: "g3", 4: "g4", 5: "b22", 6: "g6", 7: "g7"}


def _build(nc, d_in, d_h, d_out, bc):
    kt1 = d_in // 2 // P   # 8  k-tiles per L1 product
    kt2 = d_h // 2 // P    # 16 k-tiles per L2/L3 product
    rt12 = d_h // 2 // P   # 16 row tiles per L1/L2 quadrant
    rt3 = d_out // 2 // P  # 4  row tiles per L3 quadrant

    xb11_d = nc.dram_tensor("xb11", [kt1, P, NH], BF16, kind="ExternalInput")
    xb22_d = nc.dram_tensor("xb22", [2, P, 4 * NH], BF16, kind="ExternalInput")
    xc_d = {
        j: nc.dram_tensor(f"xc{j}", [P, kt1 * NH], BF16, kind="ExternalInput")
        for j in (1, 3, 4, 6, 7)
    }
    w1_d = {j: nc.dram_tensor(f"w1_{j}", [rt12, P, kt1 * P], BF16,
                              kind="ExternalInput") for j in range(1, 8)}
    w2_d = {j: nc.dram_tensor(f"w2_{j}", [rt12, P, kt2 * P], BF16,
                              kind="ExternalInput") for j in range(1, 8)}
    wo_d = {j: nc.dram_tensor(f"wo_{j}", [rt3, P, kt2 * P], BF16,
                              kind="ExternalInput") for j in range(1, 8)}
    b1t_d = nc.dram_tensor("b1t", [P, rt12], F32, kind="ExternalInput")
    b1b_d = nc.dram_tensor("b1b", [P, rt12], F32, kind="ExternalInput")
    b2t_d = nc.dram_tensor("b2t", [P, rt12], F32, kind="ExternalInput")
    b2b_d = nc.dram_tensor("b2b", [P, rt12], F32, kind="ExternalInput")
    bot_d = nc.dram_tensor("bot", [P, rt3], F32, kind="ExternalInput")
    bob_d = nc.dram_tensor("bob", [P, rt3], F32, kind="ExternalInput")
    out_d = nc.dram_tensor("out", [4 * rt3, P, NH], BF16, kind="ExternalOutput")

    with tile.TileContext(nc) as tc:
        with (
            tc.tile_pool(name="bias", bufs=1) as bias_pool,
            tc.tile_pool(name="ev", bufs=3) as ev_pool,
            tc.tile_pool(name="ch", bufs=6) as ch_pool,
            tc.tile_pool(name="ps", bufs=8, space="PSUM") as ps_pool,
        ):
            b1t = bias_pool.tile([P, rt12], F32, tag="b1t")
            b1b = bias_pool.tile([P, rt12], F32, tag="b1b")
            b2t = bias_pool.tile([P, rt12], F32, tag="b2t")
            b2b = bias_pool.tile([P, rt12], F32, tag="b2b")
            bot = bias_pool.tile([P, rt3], F32, tag="bot")
            bob = bias_pool.tile([P, rt3], F32, tag="bob")
            rings = [nc.sync, nc.scalar, nc.gpsimd]

            def emit_layer(lay, rows, kts, rhs, bt_sb, bb_sb, panel_pool,
                           panel_dram, panel_w, lookahead, prefetch_hook,
                           row_hook, out_cb):
                panels = {}
                pf = {"next": 0}
                order = [(r, jp) for r in range(rows) for jp in range(7)]

                def issue_panel(idx, split):
                    r, jp = order[idx]
                    j = PRODUCT_ORDER[jp]
                    t = panel_pool.tile([P, panel_w], BF16, tag=f"pan{lay}")
                    if split == 1:
                        rings[idx % 3].dma_start(out=t[:], in_=panel_dram[j][r])
                    else:
                        w = panel_w // split
                        for s in range(split):
                            rings[(idx + s) % 3].dma_start(
                                out=t[:, s * w:(s + 1) * w],
                                in_=panel_dram[j][r][:, s * w:(s + 1) * w],
                            )
                    panels[(r, j)] = t

                def pump(upto):
                    while pf["next"] <= min(upto, len(order) - 1):
                        issue_panel(
                            pf["next"],
                            2 if (lay == 1 and pf["next"] < 2) else 1,
                        )
                        pf["next"] += 1

                pump(0)
                for r in range(rows):
                    ps = {}
                    e = {}
                    bt = bt_sb[:, r:r + 1]
                    bb = bb_sb[:, r:r + 1]
                    for jp, j in enumerate(PRODUCT_ORDER):
                        idx = r * 7 + jp
                        pump(idx + lookahead)
                        prefetch_hook(r, jp)
                        pan = panels.pop((r, j))
                        pst = ps_pool.tile([P, NH], F32, tag="ps")
                        for kt in range(kts):
                            nc.tensor.matmul(
                                pst[:],
                                pan[:, kt * P:(kt + 1) * P],
                                rhs(j, kt),
                                start=(kt == 0),
                                stop=(kt == kts - 1),
                            )
                        ps[j] = pst
                        # combine DAG, emitted as operands become available
                        if j == 5:
                            e[5] = ev_pool.tile([P, NH], F32, name="e5", tag="ev")
                            nc.scalar.activation(e[5][:], ps[5][:], IDENT)
                        elif j == 1:
                            e[1] = ev_pool.tile([P, NH], F32, name="e1", tag="ev")
                            nc.scalar.activation(e[1][:], ps[1][:], IDENT)
                        elif j == 4:
                            e[4] = ev_pool.tile([P, NH], F32, name="e4", tag="ev")
                            nc.scalar.activation(e[4][:], ps[4][:], IDENT)
                            # C21 = M2 + M4 + bb
                            s21 = ch_pool.tile([P, NH], F32, tag="ch")
                            nc.vector.scalar_tensor_tensor(
                                s21[:], ps[2][:], bb, e[4][:], ADD, ADD)
                            out_cb("c21", r, s21)
                        elif j == 6:
                            # C22 = M1 - M2 + M3 + M6 + bb
                            d1 = ch_pool.tile([P, NH], F32, tag="ch")
                            nc.vector.scalar_tensor_tensor(
                                d1[:], ps[6][:], bb, e[1][:], ADD, ADD)
                            d2 = ch_pool.tile([P, NH], F32, tag="ch")
                            nc.vector.scalar_tensor_tensor(
                                d2[:], ps[2][:], -1.0, d1[:], MULT, ADD)
                            e["d2"] = d2
                        elif j == 3:
                            # C12 = M3 + M5 + bt
                            s12 = ch_pool.tile([P, NH], F32, tag="ch")
                            nc.vector.scalar_tensor_tensor(
                                s12[:], ps[3][:], bt, e[5][:], ADD, ADD)
                            out_cb("c12", r, s12)
                            d3 = ch_pool.tile([P, NH], F32, tag="ch")
                            nc.vector.scalar_tensor_tensor(
                                d3[:], ps[3][:], 1.0, e["d2"][:], MULT, ADD)
                            out_cb("c22", r, d3)
                        elif j == 7:
                            # C11 = M1 + M4 - M5 + M7 + bt
                            c1 = ch_pool.tile([P, NH], F32, tag="ch")
                            nc.vector.scalar_tensor_tensor(
                                c1[:], ps[7][:], bt, e[1][:], ADD, ADD)
                            c2 = ch_pool.tile([P, NH], F32, tag="ch")
                            nc.vector.scalar_tensor_tensor(
                                c2[:], e[4][:], 1.0, c1[:], MULT, ADD)
                            c3 = ch_pool.tile([P, NH], F32, tag="ch")
                            nc.vector.scalar_tensor_tensor(
                                c3[:], e[5][:], -1.0, c2[:], MULT, ADD)
                            out_cb("c11", r, c3)
                    row_hook(r)

            # =================== Layer 1 ===================
            es_h1c11 = tc.tile_pool(name="h1c11", bufs=1, side="left")
            h1c11_pool = es_h1c11.__enter__()
            es_h1c22 = tc.tile_pool(name="h1c22", bufs=1, side="left")
            h1c22_pool = es_h1c22.__enter__()
            es_g = tc.tile_pool(name="g", bufs=1, side="left")
            g_pool = es_g.__enter__()
            h1c11, h1c22 = [], []
            g = {j: [] for j in (1, 3, 4, 6, 7)}

            es_x = tc.tile_pool(name="x", bufs=1, side="left")
            x_pool = es_x.__enter__()
            es_w1 = tc.tile_pool(name="w1", bufs=4, side="left")
            w1_pool = es_w1.__enter__()
            es_h1rot = tc.tile_pool(name="h1rot", bufs=3, side="left")
            h1rot_pool = es_h1rot.__enter__()

            xb11 = [x_pool.tile([P, NH], BF16, name=f"xb11_{k}", tag=f"xb11_{k}")
                    for k in range(kt1)]
            xb22 = [x_pool.tile([P, 4 * NH], BF16, name=f"xb22_{k}", tag=f"xb22_{k}")
                    for k in range(2)]
            xc = {j: x_pool.tile([P, kt1 * NH], BF16, name=f"xc{j}", tag=f"xc{j}")
                  for j in (1, 3, 4, 6, 7)}
            nc.sync.dma_start(out=xb11[0][:], in_=xb11_d[0])
            nc.scalar.dma_start(out=xb11[1][:], in_=xb11_d[1])
            nc.gpsimd.dma_start(out=xb11[2][:], in_=xb11_d[2])
            xc_order = [1, 4, 6, 3, 7]
            st = {"xc": 0}

            def l1_prefetch(r, jp):
                if r == 0 and jp == 0:
                    for k in range(3, kt1):
                        rings[k % 3].dma_start(out=xb11[k][:], in_=xb11_d[k])
                    nc.sync.dma_start(out=xb22[0][:], in_=xb22_d[0])
                    nc.gpsimd.dma_start(out=xb22[1][:], in_=xb22_d[1])
                    nc.scalar.dma_start(out=b1t[:], in_=b1t_d[:])
                    nc.scalar.dma_start(out=b1b[:], in_=b1b_d[:])
                if r == 0 and jp < 5 and st["xc"] <= jp:
                    j = xc_order[st["xc"]]
                    rings[st["xc"] % 3].dma_start(out=xc[j][:], in_=xc_d[j][:])
                    st["xc"] += 1
                if r == 1 and jp == 0:
                    nc.scalar.dma_start(out=b2t[:], in_=b2t_d[:])
                    nc.scalar.dma_start(out=b2b[:], in_=b2b_d[:])

            def l1_rhs(j, kt):
                b = PROD_B[j]
                if b == "b11":
                    return xb11[kt][:]
                if b == "b22":
                    return xb22[kt // 4][:, (kt % 4) * NH:(kt % 4 + 1) * NH]
                return xc[j][:, kt * NH:(kt + 1) * NH]

            rowstate = {}

            def l1_out(quad, r, src):
                if quad == "c11":
                    t = h1c11_pool.tile([P, NH], BF16, name=f"h1c11_{r}",
                                        tag=f"h1c11_{r}")
                    h1c11.append(t)
                elif quad == "c22":
                    t = h1c22_pool.tile([P, NH], BF16, name=f"h1c22_{r}",
                                        tag=f"h1c22_{r}")
                    h1c22.append(t)
                else:
                    t = h1rot_pool.tile([P, NH], BF16, tag=f"h1rot_{quad}")
                    rowstate[quad] = t
                nc.vector.tensor_scalar_max(t[:], src[:], 0.0)

            def l1_row_hook(r):
                c12, c21 = rowstate["c12"], rowstate["c21"]
                c11, c22 = h1c11[r], h1c22[r]
                for j, (a, b, op) in (
                    (1, (c11, c22, ADD)), (3, (c12, c22, SUB)),
                    (4, (c21, c11, SUB)), (6, (c11, c12, ADD)),
                    (7, (c21, c22, ADD)),
                ):
                    t = g_pool.tile([P, NH], BF16, name=f"g{j}_{r}", tag=f"g{j}_{r}")
                    nc.vector.tensor_tensor(t[:], a[:], b[:], op)
                    g[j].append(t)

            emit_layer(1, rt12, kt1, l1_rhs, b1t, b1b, w1_pool, w1_d,
                       kt1 * P, 2, l1_prefetch, l1_row_hook, l1_out)

            es_h1rot.__exit__(None, None, None)
            es_w1.__exit__(None, None, None)
            es_x.__exit__(None, None, None)

            # =================== Layer 2 ===================
            es_h2c11 = tc.tile_pool(name="h2c11", bufs=1, side="right")
            h2c11_pool = es_h2c11.__enter__()
            es_h2c22 = tc.tile_pool(name="h2c22", bufs=1, side="right")
            h2c22_pool = es_h2c22.__enter__()
            es_h2c12 = tc.tile_pool(name="h2c12", bufs=1, side="right")
            h2c12_pool = es_h2c12.__enter__()
            es_h2c21 = tc.tile_pool(name="h2c21", bufs=1, side="right")
            h2c21_pool = es_h2c21.__enter__()
            es_w2 = tc.tile_pool(name="w2", bufs=3, side="left")
            w2_pool = es_w2.__enter__()
            h2 = {"c11": [], "c12": [], "c21": [], "c22": []}
            h2pools = {"c11": h2c11_pool, "c12": h2c12_pool,
                       "c21": h2c21_pool, "c22": h2c22_pool}

            def l2_rhs(j, kt):
                b = PROD_B[j]
                if b == "b11":
                    return h1c11[kt][:]
                if b == "b22":
                    return h1c22[kt][:]
                return g[j][kt][:]

            def l2_out(quad, r, src):
                t = h2pools[quad].tile([P, NH], BF16, name=f"h2{quad}_{r}",
                                       tag=f"h2{quad}_{r}")
                nc.vector.tensor_scalar_max(t[:], src[:], 0.0)
                h2[quad].append(t)

            emit_layer(2, rt12, kt2, l2_rhs, b2t, b2b, w2_pool, w2_d,
                       kt2 * P, 1, lambda r, jp: None, lambda r: None, l2_out)

            es_w2.__exit__(None, None, None)
            es_g.__exit__(None, None, None)
            es_h1c22.__exit__(None, None, None)
            es_h1c11.__exit__(None, None, None)

            # =================== Layer 3 ===================
            # Phase A: the 8 raw-B products (M2/M5 of all 4 rows) run first,
            # each evicted to SBUF f32 by the scalar engine on completion;
            # all 80 h2 B-combos build on the vector engine under that
            # ~28us of PE cover (gpsimd tensor_tensor measured ~1.4us/tile,
            # 4x slower than DVE -- keep it DMA-only).  Phase B: the 5
            # combo products per row; combines read e2/e5 from SBUF so every
            # STT still has exactly one PSUM operand.
            es_d = tc.tile_pool(name="d", bufs=1, side="right")
            d_pool = es_d.__enter__()
            es_wo = tc.tile_pool(name="wo", bufs=4, side="right")
            wo_pool = es_wo.__enter__()
            es_osb = tc.tile_pool(name="osb", bufs=8, side="right")
            osb_pool = es_osb.__enter__()
            es_evA = tc.tile_pool(name="evA", bufs=1, side="right")
            evA_pool = es_evA.__enter__()
            d = {j: [] for j in (1, 3, 4, 6, 7)}
            d_specs = []
            for j, (qa, qb, op) in (
                (1, ("c11", "c22", ADD)), (4, ("c21", "c11", SUB)),
                (6, ("c11", "c12", ADD)), (3, ("c12", "c22", SUB)),
                (7, ("c21", "c22", ADD)),
            ):
                for kt in range(kt2):
                    d_specs.append((j, kt, qa, qb, op))
            dst = {"next": 0}

            def build_d(n):
                while dst["next"] < min(n, len(d_specs)):
                    j, kt, qa, qb, op = d_specs[dst["next"]]
                    t = d_pool.tile([P, NH], BF16, name=f"d{j}_{kt}",
                                    tag=f"d{j}_{kt}")
                    nc.vector.tensor_tensor(t[:], h2[qa][kt][:], h2[qb][kt][:], op)
                    d[j].append(t)
                    dst["next"] += 1

            oq = {"c11": 0, "c12": 1, "c21": 2, "c22": 3}
            ost = {"n": 0}

            def l3_out(quad, r, src):
                mo = oq[quad] * rt3 + r
                if r == rt3 - 1:
                    hw = NH // 2
                    nc.sync.dma_start(out=out_d[mo][:, 0:hw], in_=src[:, 0:hw])
                    nc.scalar.dma_start(out=out_d[mo][:, hw:], in_=src[:, hw:])
                else:
                    eng = nc.sync if ost["n"] % 2 else nc.scalar
                    eng.dma_start(out=out_d[mo], in_=src[:])
                    ost["n"] += 1

            orderA = [(r, 2) for r in range(rt3)] + [(r, 5) for r in range(rt3)]
            orderB = [(r, j) for r in range(rt3) for j in (1, 4, 6, 3, 7)]
            order3 = orderA + orderB
            panels3 = {}
            pf3 = {"next": 0}

            def pump3(upto):
                while pf3["next"] <= min(upto, len(order3) - 1):
                    idx = pf3["next"]
                    r3, j3 = order3[idx]
                    t = wo_pool.tile([P, kt2 * P], BF16, tag="pan3")
                    rings[idx % 3].dma_start(out=t[:], in_=wo_d[j3][r3])
                    panels3[(r3, j3)] = t
                    pf3["next"] += 1

            def l3_mm(r, j, rhs_fn):
                pst = ps_pool.tile([P, NH], F32, tag="ps")
                pan = panels3.pop((r, j))
                for kt in range(kt2):
                    nc.tensor.matmul(
                        pst[:],
                        pan[:, kt * P:(kt + 1) * P],
                        rhs_fn(kt),
                        start=(kt == 0),
                        stop=(kt == kt2 - 1),
                    )
                return pst

            pump3(1)
            e25 = {}
            for ai, (r, j) in enumerate(orderA):
                pump3(ai + 2)
                if ai == 0:
                    nc.scalar.dma_start(out=bot[:], in_=bot_d[:])
                    nc.scalar.dma_start(out=bob[:], in_=bob_d[:])
                src = h2["c11"] if j == 2 else h2["c22"]
                pst = l3_mm(r, j, lambda kt: src[kt][:])
                ev = evA_pool.tile([P, NH], F32, name=f"eA{j}_{r}", tag=f"eA{j}_{r}")
                nc.scalar.activation(ev[:], pst[:], IDENT)
                e25[(j, r)] = ev
                build_d(10 * (ai + 1))
            build_d(len(d_specs))

            ps1 = c1 = c2 = d2 = None
            for bi, (r, j) in enumerate(orderB):
                pump3(len(orderA) + bi + 2)
                bt = bot[:, r:r + 1]
                bb = bob[:, r:r + 1]
                pst = l3_mm(r, j, lambda kt: d[j][kt][:])
                e2, e5 = e25[(2, r)], e25[(5, r)]
                if j == 1:
                    ps1 = pst
                    c1 = ch_pool.tile([P, NH], F32, tag="ch")
                    nc.vector.scalar_tensor_tensor(c1[:], pst[:], bt, e5[:], ADD, SUB)
                elif j == 4:
                    o21 = osb_pool.tile([P, NH], BF16, tag="osb")
                    nc.vector.scalar_tensor_tensor(o21[:], pst[:], bb, e2[:], ADD, ADD)
                    l3_out("c21", r, o21)
                    c2 = ch_pool.tile([P, NH], F32, tag="ch")
                    nc.vector.scalar_tensor_tensor(c2[:], pst[:], 1.0, c1[:], MULT, ADD)
                elif j == 6:
                    d1 = ch_pool.tile([P, NH], F32, tag="ch")
                    nc.vector.scalar_tensor_tensor(d1[:], pst[:], bb, e2[:], ADD, SUB)
                    d2 = ch_pool.tile([P, NH], F32, tag="ch")
                    nc.vector.scalar_tensor_tensor(d2[:], ps1[:], 1.0, d1[:], MULT, ADD)
                elif j == 3:
                    o12 = osb_pool.tile([P, NH], BF16, tag="osb")
                    nc.vector.scalar_tensor_tensor(o12[:], pst[:], bt, e5[:], ADD, ADD)
                    l3_out("c12", r, o12)
                    o22 = osb_pool.tile([P, NH], BF16, tag="osb")
                    nc.vector.scalar_tensor_tensor(o22[:], pst[:], 1.0, d2[:], MULT, ADD)
                    l3_out("c22", r, o22)
                elif j == 7:
                    o11 = osb_pool.tile([P, NH], BF16, tag="osb")
                    nc.vector.scalar_tensor_tensor(o11[:], pst[:], 1.0, c2[:], MULT, ADD)
                    l3_out("c11", r, o11)

            es_evA.__exit__(None, None, None)
            es_osb.__exit__(None, None, None)
            es_wo.__exit__(None, None, None)
            es_d.__exit__(None, None, None)
            es_h2c21.__exit__(None, None, None)
            es_h2c12.__exit__(None, None, None)
            es_h2c22.__exit__(None, None, None)
            es_h2c11.__exit__(None, None, None)

    nc.compile()
    return nc


def _expand_mask(mask, t=TILE):
    return np.repeat(np.repeat(np.asarray(mask, dtype=bool), t, axis=0), t, axis=1)


def _pack_lhsT(w):
    """[d_m, d_k] -> [d_m/P, P, d_k] lhsT panels (partition = contraction)."""
    d_m, d_k = w.shape
    mt, kt = d_m // P, d_k // P
    return np.ascontiguousarray(
        w.reshape(mt, P, kt, P).transpose(0, 3, 2, 1).reshape(mt, P, d_k)
    )


def _strassen_a(w):
    m, k = w.shape
    mh, kh = m // 2, k // 2
    A11, A12 = w[:mh, :kh], w[:mh, kh:]
    A21, A22 = w[mh:, :kh], w[mh:, kh:]
    return {
        1: A11 + A22, 2: A21 + A22, 3: A11, 4: A22,
        5: A11 + A12, 6: A21 - A11, 7: A12 - A22,
    }


def _pack_bias(b):
    n = b.shape[0] // P
    return np.ascontiguousarray(b.reshape(n, P).T)


def _run(x, w1e, b1, w2e, b2, wo, bo, d_in, d_h, d_out, n_cores=N_CORES, trace=False):
    b = x.shape[0]
    bc = b // n_cores
    assert bc == 2 * NH

    nc = bacc.Bacc("TRN2", target_bir_lowering=False, debug=False, num_devices=n_cores)
    _build(nc, d_in, d_h, d_out, bc)

    np_bf16 = mybir.dt.np(BF16)

    def cvt(a):
        return np.ascontiguousarray(a.astype(np_bf16))

    shared = {}
    for name, w in (("w1", w1e), ("w2", w2e), ("wo", wo)):
        for j, a in _strassen_a(w).items():
            shared[f"{name}_{j}"] = cvt(_pack_lhsT(a))
    shared["b1t"] = _pack_bias(b1[:d_h // 2])
    shared["b1b"] = _pack_bias(b1[d_h // 2:])
    shared["b2t"] = _pack_bias(b2[:d_h // 2])
    shared["b2b"] = _pack_bias(b2[d_h // 2:])
    shared["bot"] = _pack_bias(bo[:d_out // 2])
    shared["bob"] = _pack_bias(bo[d_out // 2:])

    kh1 = d_in // 2
    kt1 = kh1 // P
    in_maps = []
    for c in range(n_cores):
        xc_ = np.ascontiguousarray(x[c * bc:(c + 1) * bc].T)  # [d_in, bc]
        B11 = xc_[:kh1, :NH]
        B12 = xc_[:kh1, NH:]
        B21 = xc_[kh1:, :NH]
        B22 = xc_[kh1:, NH:]
        m = {
            "xb11": cvt(B11.reshape(kt1, P, NH)),
            "xb22": cvt(
                B22.reshape(2, 4, P, NH).transpose(0, 2, 1, 3).reshape(2, P, 4 * NH)
            ),
        }
        for j, comb in (
            (1, B11 + B22), (3, B12 - B22), (4, B21 - B11),
            (6, B11 + B12), (7, B21 + B22),
        ):
            m[f"xc{j}"] = cvt(
                comb.reshape(kt1, P, NH).transpose(1, 0, 2).reshape(P, kt1 * NH)
            )
        in_maps.append({**m, **shared})

    res = run_bass_kernel_spmd(nc, in_maps, core_ids=list(range(n_cores)), trace=trace)
    outs = []
    rt3 = d_out // 2 // P
    for c in range(n_cores):
        o = res.results[c]["out"].reshape(4, rt3 * P, NH).astype(np.float32)
        full = np.empty((d_out, bc), np.float32)
        full[:rt3 * P, :NH] = o[0]
        full[:rt3 * P, NH:] = o[1]
        full[rt3 * P:, :NH] = o[2]
        full[rt3 * P:, NH:] = o[3]
        outs.append(full)
    full = np.concatenate(outs, axis=1)  # [d_out, B]
    return np.ascontiguousarray(full.T), res


def kernel(x, W1, b1, W2, b2, Wo, bo, mask1, mask2):
    x = np.asarray(x, dtype=np.float32)
    w1e = np.asarray(W1, dtype=np.float32) * _expand_mask(mask1)
    w2e = np.asarray(W2, dtype=np.float32) * _expand_mask(mask2)
    out, _ = _run(
        x,
        w1e,
        np.asarray(b1, np.float32),
        w2e,
        np.asarray(b2, np.float32),
        np.asarray(Wo, np.float32),
        np.asarray(bo, np.float32),
        d_in=2048,
        d_h=4096,
        d_out=1024,
    )
    return out


# revision 11
# speedup vs baseline: 1.1101x; 1.0010x over previous
"""Block-sparse 3-layer MLP on 8 Trainium2 NeuronCores, via 1-level Strassen.

Reference computation (fp32):
    h1 = relu(x @ (W1*expand(mask1)).T + b1)       x:[B,2048] W1:[4096,2048]
    h2 = relu(h1 @ (W2*expand(mask2)).T + b2)      W2:[4096,4096]
    out = h2 @ Wo.T + bo                           Wo:[1024,4096] -> [B,1024]

Strategy: data-parallel over the batch (B=8192 -> bc=1024 per core), no
collectives, feature-major activations [features, batch].  The masks make the
weights block-sparse (32x32 tiles, i.i.d. 0.5 density) but i.i.d. 32-granular
sparsity cannot beat dense on a 128x128 PE (any 4-row/4-col-block packing is
~94% dense by the union bound) and fp8 fails the 2e-2 gate (measured 6.2e-2
one-pass, 4.4e-2 with a 2-term split).  Dense bf16 streams at 216ns per
[128x128]x[128x512] matmul and the dense baseline already ran at 95.4% PE
occupancy, so the remaining lever is cutting PE work itself:

Each layer h = W.x runs 1-level Strassen on the 2x2 block split of W
([m/2,k/2] quadrants) and of the feature-major activation ([k/2, 512]
quadrants; the 1024 batch splits into two 512 halves):
    M1=(A11+A22)(B11+B22) M2=(A21+A22)B11 M3=A11(B12-B22) M4=A22(B21-B11)
    M5=(A11+A12)B22 M6=(A21-A11)(B11+B12) M7=(A12-A22)(B21+B22)
    C11=M1+M4-M5+M7 C12=M3+M5 C21=M2+M4 C22=M1-M2+M3+M6
7 half-size products instead of 8: 3136 matmuls/core vs 3584 dense (PE floor
677us vs 773us).  Measured numeric cost: 7.5e-3 rel err vs 3.9e-3 dense bf16.

- A-side combos are free (host precomputes 7 bf16 lhsT panel sets per layer;
  1.75x weight HBM bytes ~ 103MB/core ~ 150GB/s sustained, well within ring
  fanout).  x-side B combos are host-computed too (x is an input).
- h1's B-combos for L2 are built on-device per row tile (5 bf16
  tensor_tensor adds on gpsimd, overlapped with products); h1's C12/C21
  quadrant tiles are freed right after (rotating pool) since only C11/C22
  are consumed raw (M2/M5).  h2's combos cannot coexist with h1's in SBUF,
  so they are deferred to L3 start and built just-in-time under L3 row 0's
  M2/M5 products (which read raw h2 quadrants), split across gpsimd+vector.
- C-combines run on the vector engine as scalar_tensor_tensor chains
  (out=(in0 op0 scalar) op1 in1), each reading exactly one PSUM operand (ISA
  limit) plus one SBUF f32 partial.  M1/M4/M5 are evicted to SBUF by the
  scalar engine (activation IDENT); bias rides the STT scalar slot; relu is
  a vector tensor_scalar_max into the resident bf16 h tiles.  In L3 the
  final STT of each quadrant writes the bf16 output tile directly.
- PSUM: every product accumulates over k/2 into its own [128,512] f32 tile
  (exactly one PSUM bank); peak ~4 live banks per row, one 8-deep pool
  rotates across rows and layers without PE stalls.
- lhsT panels stream per-product on the three DMA rings (sync/scalar/
  gpsimd), 1-2 products ahead; row-0 panels and the first x tiles are split
  finer so the first matmul issues ~2us in.
"""

import sys

sys.path.insert(0, "/opt/trn_rl_repo")

import numpy as np

from concourse import bacc, mybir, tile
from concourse.bass_utils import run_bass_kernel_spmd

F32 = mybir.dt.float32
BF16 = mybir.dt.bfloat16
IDENT = mybir.ActivationFunctionType.Identity
ADD = mybir.AluOpType.add
SUB = mybir.AluOpType.subtract
MULT = mybir.AluOpType.mult

N_CORES = 8
TILE = 32
P = 128
NH = 512  # half-batch strip = one psum bank

# per-row product order: raw-B products first (startup / lazy-combo cover),
# early-evicted M1 next, then combine-dependency order
PRODUCT_ORDER = [2, 5, 1, 4, 6, 3, 7]
PRODUCT_ORDER_LAST = [2, 5, 1, 4, 6, 7, 3]
PROD_B = {1: "g1", 2: "b11", 3: "g3", 4: "g4", 5: "b22", 6: "g6", 7: "g7"}


def _build(nc, d_in, d_h, d_out, bc):
    kt1 = d_in // 2 // P   # 8  k-tiles per L1 product
    kt2 = d_h // 2 // P    # 16 k-tiles per L2/L3 product
    rt12 = d_h // 2 // P   # 16 row tiles per L1/L2 quadrant
    rt3 = d_out // 2 // P  # 4  row tiles per L3 quadrant

    xb11_d = nc.dram_tensor("xb11", [kt1, P, NH], BF16, kind="ExternalInput")
    xb22_d = nc.dram_tensor("xb22", [2, P, 4 * NH], BF16, kind="ExternalInput")
    xc_d = {
        j: nc.dram_tensor(f"xc{j}", [P, kt1 * NH], BF16, kind="ExternalInput")
        for j in (1, 3, 4, 6, 7)
    }
    w1_d = {j: nc.dram_tensor(f"w1_{j}", [rt12, P, kt1 * P], BF16,
                              kind="ExternalInput") for j in range(1, 8)}
    w2_d = {j: nc.dram_tensor(f"w2_{j}", [rt12, P, kt2 * P], BF16,
                              kind="ExternalInput") for j in range(1, 8)}
    wo_d = {j: nc.dram_tensor(f"wo_{j}", [rt3, P, kt2 * P], BF16,
                              kind="ExternalInput") for j in range(1, 8)}
    b1t_d = nc.dram_tensor("b1t", [P, rt12], F32, kind="ExternalInput")
    b1b_d = nc.dram_tensor("b1b", [P, rt12], F32, kind="ExternalInput")
    b2t_d = nc.dram_tensor("b2t", [P, rt12], F32, kind="ExternalInput")
    b2b_d = nc.dram_tensor("b2b", [P, rt12], F32, kind="ExternalInput")
    bot_d = nc.dram_tensor("bot", [P, rt3], F32, kind="ExternalInput")
    bob_d = nc.dram_tensor("bob", [P, rt3], F32, kind="ExternalInput")
    out_d = nc.dram_tensor("out", [4 * rt3, P, NH], BF16, kind="ExternalOutput")

    with tile.TileContext(nc) as tc:
        with (
            tc.tile_pool(name="bias", bufs=1) as bias_pool,
            tc.tile_pool(name="ev", bufs=3) as ev_pool,
            tc.tile_pool(name="ch", bufs=6) as ch_pool,
            tc.tile_pool(name="ps", bufs=8, space="PSUM") as ps_pool,
        ):
            b1t = bias_pool.tile([P, rt12], F32, tag="b1t")
            b1b = bias_pool.tile([P, rt12], F32, tag="b1b")
            b2t = bias_pool.tile([P, rt12], F32, tag="b2t")
            b2b = bias_pool.tile([P, rt12], F32, tag="b2b")
            bot = bias_pool.tile([P, rt3], F32, tag="bot")
            bob = bias_pool.tile([P, rt3], F32, tag="bob")
            rings = [nc.sync, nc.scalar, nc.gpsimd]

            def emit_layer(lay, rows, kts, rhs, bt_sb, bb_sb, panel_pool,
                           panel_dram, panel_w, lookahead, prefetch_hook,
                           row_hook, out_cb):
                panels = {}
                pf = {"next": 0}
                def row_order(r):
                    return PRODUCT_ORDER_LAST if r == rows - 1 else PRODUCT_ORDER

                order = [(r, jp) for r in range(rows) for jp in range(7)]

                def issue_panel(idx, split):
                    r, jp = order[idx]
                    j = row_order(r)[jp]
                    t = panel_pool.tile([P, panel_w], BF16, tag=f"pan{lay}")
                    if split == 1:
                        rings[idx % 3].dma_start(out=t[:], in_=panel_dram[j][r])
                    else:
                        w = panel_w // split
                        for s in range(split):
                            rings[(idx + s) % 3].dma_start(
                                out=t[:, s * w:(s + 1) * w],
                                in_=panel_dram[j][r][:, s * w:(s + 1) * w],
                            )
                    panels[(r, j)] = t

                def pump(upto):
                    while pf["next"] <= min(upto, len(order) - 1):
                        issue_panel(
                            pf["next"],
                            4 if (lay == 1 and pf["next"] < 2) else 1,
                        )
                        pf["next"] += 1

                pump(0)
                for r in range(rows):
                    ps = {}
                    e = {}
                    bt = bt_sb[:, r:r + 1]
                    bb = bb_sb[:, r:r + 1]
                    for jp, j in enumerate(row_order(r)):
                        idx = r * 7 + jp
                        pump(idx + lookahead)
                        prefetch_hook(r, jp)
                        pan = panels.pop((r, j))
                        pst = ps_pool.tile([P, NH], F32, tag="ps")
                        for kt in range(kts):
                            nc.tensor.matmul(
                                pst[:],
                                pan[:, kt * P:(kt + 1) * P],
                                rhs(j, kt),
                                start=(kt == 0),
                                stop=(kt == kts - 1),
                            )
                        ps[j] = pst
                        # combine DAG, emitted as operands become available
                        if j == 5:
                            e[5] = ev_pool.tile([P, NH], F32, name="e5", tag="ev")
                            nc.scalar.activation(e[5][:], ps[5][:], IDENT)
                        elif j == 1:
                            e[1] = ev_pool.tile([P, NH], F32, name="e1", tag="ev")
                            nc.scalar.activation(e[1][:], ps[1][:], IDENT)
                        elif j == 4:
                            e[4] = ev_pool.tile([P, NH], F32, name="e4", tag="ev")
                            nc.scalar.activation(e[4][:], ps[4][:], IDENT)
                            # C21 = M2 + M4 + bb
                            s21 = ch_pool.tile([P, NH], F32, tag="ch")
                            nc.vector.scalar_tensor_tensor(
                                s21[:], ps[2][:], bb, e[4][:], ADD, ADD)
                            out_cb("c21", r, s21)
                        elif j == 6:
                            # C22 = M1 - M2 + M3 + M6 + bb
                            d1 = ch_pool.tile([P, NH], F32, tag="ch")
                            nc.vector.scalar_tensor_tensor(
                                d1[:], ps[6][:], bb, e[1][:], ADD, ADD)
                            d2 = ch_pool.tile([P, NH], F32, tag="ch")
                            nc.vector.scalar_tensor_tensor(
                                d2[:], ps[2][:], -1.0, d1[:], MULT, ADD)
                            e["d2"] = d2
                        elif j == 3:
                            # C12 = M3 + M5 + bt
                            s12 = ch_pool.tile([P, NH], F32, tag="ch")
                            nc.vector.scalar_tensor_tensor(
                                s12[:], ps[3][:], bt, e[5][:], ADD, ADD)
                            out_cb("c12", r, s12)
                            d3 = ch_pool.tile([P, NH], F32, tag="ch")
                            nc.vector.scalar_tensor_tensor(
                                d3[:], ps[3][:], 1.0, e["d2"][:], MULT, ADD)
                            out_cb("c22", r, d3)
                        elif j == 7:
                            # C11 = M1 + M4 - M5 + M7 + bt
                            c1 = ch_pool.tile([P, NH], F32, tag="ch")
                            nc.vector.scalar_tensor_tensor(
                                c1[:], ps[7][:], bt, e[1][:], ADD, ADD)
                            c2 = ch_pool.tile([P, NH], F32, tag="ch")
                            nc.vector.scalar_tensor_tensor(
                                c2[:], e[4][:], 1.0, c1[:], MULT, ADD)
                            c3 = ch_pool.tile([P, NH], F32, tag="ch")
                            nc.vector.scalar_tensor_tensor(
                                c3[:], e[5][:], -1.0, c2[:], MULT, ADD)
                            out_cb("c11", r, c3)
                    row_hook(r)

            # =================== Layer 1 ===================
            es_h1c11 = tc.tile_pool(name="h1c11", bufs=1, side="left")
            h1c11_pool = es_h1c11.__enter__()
            es_h1c22 = tc.tile_pool(name="h1c22", bufs=1, side="left")
            h1c22_pool = es_h1c22.__enter__()
            es_g = tc.tile_pool(name="g", bufs=1, side="left")
            g_pool = es_g.__enter__()
            h1c11, h1c22 = [], []
            g = {j: [] for j in (1, 3, 4, 6, 7)}

            es_x = tc.tile_pool(name="x", bufs=1, side="left")
            x_pool = es_x.__enter__()
            es_w1 = tc.tile_pool(name="w1", bufs=4, side="left")
            w1_pool = es_w1.__enter__()
            es_h1rot = tc.tile_pool(name="h1rot", bufs=3, side="left")
            h1rot_pool = es_h1rot.__enter__()

            xb11 = [x_pool.tile([P, NH], BF16, name=f"xb11_{k}", tag=f"xb11_{k}")
                    for k in range(kt1)]
            xb22 = [x_pool.tile([P, 4 * NH], BF16, name=f"xb22_{k}", tag=f"xb22_{k}")
                    for k in range(2)]
            xc = {j: x_pool.tile([P, kt1 * NH], BF16, name=f"xc{j}", tag=f"xc{j}")
                  for j in (1, 3, 4, 6, 7)}
            nc.sync.dma_start(out=xb11[0][:], in_=xb11_d[0])
            nc.scalar.dma_start(out=xb11[1][:], in_=xb11_d[1])
            nc.gpsimd.dma_start(out=xb11[2][:], in_=xb11_d[2])

            def half(eng, t, dsrc, s):
                hw = t.shape[1] // 2
                eng.dma_start(out=t[:, s * hw:(s + 1) * hw],
                              in_=dsrc[:, s * hw:(s + 1) * hw])

            def l1_prefetch(r, jp):
                # need order: xb22 (jp1), xc1 (jp2), xc4 (jp3), xc6, xc3, xc7;
                # halves spread over the three rings, all in flight by jp3
                if r == 0 and jp == 0:
                    for k in range(3, kt1):
                        rings[k % 3].dma_start(out=xb11[k][:], in_=xb11_d[k])
                    half(nc.sync, xb22[0], xb22_d[0], 0)
                    half(nc.gpsimd, xb22[0], xb22_d[0], 1)
                    half(nc.scalar, xc[1], xc_d[1], 0)
                    half(nc.sync, xc[1], xc_d[1], 1)
                elif r == 0 and jp == 1:
                    half(nc.gpsimd, xc[4], xc_d[4], 0)
                    half(nc.scalar, xc[4], xc_d[4], 1)
                    half(nc.sync, xb22[1], xb22_d[1], 0)
                    half(nc.gpsimd, xb22[1], xb22_d[1], 1)
                elif r == 0 and jp == 2:
                    half(nc.scalar, xc[6], xc_d[6], 0)
                    half(nc.sync, xc[6], xc_d[6], 1)
                    half(nc.gpsimd, xc[3], xc_d[3], 0)
                    half(nc.scalar, xc[3], xc_d[3], 1)
                    nc.scalar.dma_start(out=b1t[:], in_=b1t_d[:])
                    nc.scalar.dma_start(out=b1b[:], in_=b1b_d[:])
                elif r == 0 and jp == 3:
                    half(nc.sync, xc[7], xc_d[7], 0)
                    half(nc.gpsimd, xc[7], xc_d[7], 1)
                elif r == 1 and jp == 0:
                    nc.scalar.dma_start(out=b2t[:], in_=b2t_d[:])
                    nc.scalar.dma_start(out=b2b[:], in_=b2b_d[:])

            def l1_rhs(j, kt):
                b = PROD_B[j]
                if b == "b11":
                    return xb11[kt][:]
                if b == "b22":
                    return xb22[kt // 4][:, (kt % 4) * NH:(kt % 4 + 1) * NH]
                return xc[j][:, kt * NH:(kt + 1) * NH]

            rowstate = {}

            def l1_out(quad, r, src):
                if quad == "c11":
                    t = h1c11_pool.tile([P, NH], BF16, name=f"h1c11_{r}",
                                        tag=f"h1c11_{r}")
                    h1c11.append(t)
                elif quad == "c22":
                    t = h1c22_pool.tile([P, NH], BF16, name=f"h1c22_{r}",
                                        tag=f"h1c22_{r}")
                    h1c22.append(t)
                else:
                    t = h1rot_pool.tile([P, NH], BF16, tag=f"h1rot_{quad}")
                    rowstate[quad] = t
                nc.vector.tensor_scalar_max(t[:], src[:], 0.0)

            def l1_row_hook(r):
                c12, c21 = rowstate["c12"], rowstate["c21"]
                c11, c22 = h1c11[r], h1c22[r]
                for j, (a, b, op) in (
                    (1, (c11, c22, ADD)), (3, (c12, c22, SUB)),
                    (4, (c21, c11, SUB)), (6, (c11, c12, ADD)),
                    (7, (c21, c22, ADD)),
                ):
                    t = g_pool.tile([P, NH], BF16, name=f"g{j}_{r}", tag=f"g{j}_{r}")
                    nc.vector.tensor_tensor(t[:], a[:], b[:], op)
                    g[j].append(t)

            emit_layer(1, rt12, kt1, l1_rhs, b1t, b1b, w1_pool, w1_d,
                       kt1 * P, 2, l1_prefetch, l1_row_hook, l1_out)

            es_h1rot.__exit__(None, None, None)
            es_w1.__exit__(None, None, None)
            es_x.__exit__(None, None, None)

            # =================== Layer 2 ===================
            es_h2c11 = tc.tile_pool(name="h2c11", bufs=1, side="right")
            h2c11_pool = es_h2c11.__enter__()
            es_h2c22 = tc.tile_pool(name="h2c22", bufs=1, side="right")
            h2c22_pool = es_h2c22.__enter__()
            es_h2c12 = tc.tile_pool(name="h2c12", bufs=1, side="right")
            h2c12_pool = es_h2c12.__enter__()
            es_h2c21 = tc.tile_pool(name="h2c21", bufs=1, side="right")
            h2c21_pool = es_h2c21.__enter__()
            es_w2 = tc.tile_pool(name="w2", bufs=3, side="left")
            w2_pool = es_w2.__enter__()
            h2 = {"c11": [], "c12": [], "c21": [], "c22": []}
            h2pools = {"c11": h2c11_pool, "c12": h2c12_pool,
                       "c21": h2c21_pool, "c22": h2c22_pool}

            def l2_rhs(j, kt):
                b = PROD_B[j]
                if b == "b11":
                    return h1c11[kt][:]
                if b == "b22":
                    return h1c22[kt][:]
                return g[j][kt][:]

            def l2_out(quad, r, src):
                t = h2pools[quad].tile([P, NH], BF16, name=f"h2{quad}_{r}",
                                       tag=f"h2{quad}_{r}")
                nc.vector.tensor_scalar_max(t[:], src[:], 0.0)
                h2[quad].append(t)

            emit_layer(2, rt12, kt2, l2_rhs, b2t, b2b, w2_pool, w2_d,
                       kt2 * P, 1, lambda r, jp: None, lambda r: None, l2_out)

            es_w2.__exit__(None, None, None)
            es_g.__exit__(None, None, None)
            es_h1c22.__exit__(None, None, None)
            es_h1c11.__exit__(None, None, None)

            # =================== Layer 3 ===================
            # Phase A: the 8 raw-B products (M2/M5 of all 4 rows) run first,
            # each evicted to SBUF f32 by the scalar engine on completion;
            # all 80 h2 B-combos build on the vector engine under that
            # ~28us of PE cover (gpsimd tensor_tensor measured ~1.4us/tile,
            # 4x slower than DVE -- keep it DMA-only).  Phase B: the 5
            # combo products per row; combines read e2/e5 from SBUF so every
            # STT still has exactly one PSUM operand.
            es_d = tc.tile_pool(name="d", bufs=1, side="right")
            d_pool = es_d.__enter__()
            es_wo = tc.tile_pool(name="wo", bufs=4, side="right")
            wo_pool = es_wo.__enter__()
            es_osb = tc.tile_pool(name="osb", bufs=8, side="right")
            osb_pool = es_osb.__enter__()
            es_evA = tc.tile_pool(name="evA", bufs=1, side="right")
            evA_pool = es_evA.__enter__()
            d = {j: [] for j in (1, 3, 4, 6, 7)}
            d_specs = []
            for j, (qa, qb, op) in (
                (1, ("c11", "c22", ADD)), (4, ("c21", "c11", SUB)),
                (6, ("c11", "c12", ADD)), (3, ("c12", "c22", SUB)),
                (7, ("c21", "c22", ADD)),
            ):
                for kt in range(kt2):
                    d_specs.append((j, kt, qa, qb, op))
            dst = {"next": 0}

            def build_d(n):
                while dst["next"] < min(n, len(d_specs)):
                    j, kt, qa, qb, op = d_specs[dst["next"]]
                    t = d_pool.tile([P, NH], BF16, name=f"d{j}_{kt}",
                                    tag=f"d{j}_{kt}")
                    nc.vector.tensor_tensor(t[:], h2[qa][kt][:], h2[qb][kt][:], op)
                    d[j].append(t)
                    dst["next"] += 1

            oq = {"c11": 0, "c12": 1, "c21": 2, "c22": 3}
            ost = {"n": 0}

            def l3_out(quad, r, src):
                mo = oq[quad] * rt3 + r
                if r == rt3 - 1:
                    hw = NH // 2
                    nc.sync.dma_start(out=out_d[mo][:, 0:hw], in_=src[:, 0:hw])
                    nc.scalar.dma_start(out=out_d[mo][:, hw:], in_=src[:, hw:])
                else:
                    eng = nc.sync if ost["n"] % 2 else nc.scalar
                    eng.dma_start(out=out_d[mo], in_=src[:])
                    ost["n"] += 1

            orderA = [(r, 2) for r in range(rt3)] + [(r, 5) for r in range(rt3)]
            orderB = [(r, j) for r in range(rt3) for j in (1, 4, 6, 3, 7)]
            order3 = orderA + orderB
            panels3 = {}
            pf3 = {"next": 0}

            def pump3(upto):
                while pf3["next"] <= min(upto, len(order3) - 1):
                    idx = pf3["next"]
                    r3, j3 = order3[idx]
                    t = wo_pool.tile([P, kt2 * P], BF16, tag="pan3")
                    rings[idx % 3].dma_start(out=t[:], in_=wo_d[j3][r3])
                    panels3[(r3, j3)] = t
                    pf3["next"] += 1

            def l3_mm(r, j, rhs_fn):
                pst = ps_pool.tile([P, NH], F32, tag="ps")
                pan = panels3.pop((r, j))
                for kt in range(kt2):
                    nc.tensor.matmul(
                        pst[:],
                        pan[:, kt * P:(kt + 1) * P],
                        rhs_fn(kt),
                        start=(kt == 0),
                        stop=(kt == kt2 - 1),
                    )
                return pst

            pump3(1)
            e25 = {}
            for ai, (r, j) in enumerate(orderA):
                pump3(ai + 2)
                if ai == 0:
                    nc.scalar.dma_start(out=bot[:], in_=bot_d[:])
                    nc.scalar.dma_start(out=bob[:], in_=bob_d[:])
                src = h2["c11"] if j == 2 else h2["c22"]
                pst = l3_mm(r, j, lambda kt: src[kt][:])
                ev = evA_pool.tile([P, NH], F32, name=f"eA{j}_{r}", tag=f"eA{j}_{r}")
                nc.scalar.activation(ev[:], pst[:], IDENT)
                e25[(j, r)] = ev
                build_d(10 * (ai + 1))
            build_d(len(d_specs))

            ps1 = c1 = c2 = d2 = None
            for bi, (r, j) in enumerate(orderB):
                pump3(len(orderA) + bi + 2)
                bt = bot[:, r:r + 1]
                bb = bob[:, r:r + 1]
                pst = l3_mm(r, j, lambda kt: d[j][kt][:])
                e2, e5 = e25[(2, r)], e25[(5, r)]
                if j == 1:
                    ps1 = pst
                    c1 = ch_pool.tile([P, NH], F32, tag="ch")
                    nc.vector.scalar_tensor_tensor(c1[:], pst[:], bt, e5[:], ADD, SUB)
                elif j == 4:
                    o21 = osb_pool.tile([P, NH], BF16, tag="osb")
                    nc.vector.scalar_tensor_tensor(o21[:], pst[:], bb, e2[:], ADD, ADD)
                    l3_out("c21", r, o21)
                    c2 = ch_pool.tile([P, NH], F32, tag="ch")
                    nc.vector.scalar_tensor_tensor(c2[:], pst[:], 1.0, c1[:], MULT, ADD)
                elif j == 6:
                    d1 = ch_pool.tile([P, NH], F32, tag="ch")
                    nc.vector.scalar_tensor_tensor(d1[:], pst[:], bb, e2[:], ADD, SUB)
                    d2 = ch_pool.tile([P, NH], F32, tag="ch")
                    nc.vector.scalar_tensor_tensor(d2[:], ps1[:], 1.0, d1[:], MULT, ADD)
                elif j == 3:
                    o12 = osb_pool.tile([P, NH], BF16, tag="osb")
                    nc.vector.scalar_tensor_tensor(o12[:], pst[:], bt, e5[:], ADD, ADD)
                    l3_out("c12", r, o12)
                    o22 = osb_pool.tile([P, NH], BF16, tag="osb")
                    nc.vector.scalar_tensor_tensor(o22[:], pst[:], 1.0, d2[:], MULT, ADD)
                    l3_out("c22", r, o22)
                elif j == 7:
                    o11 = osb_pool.tile([P, NH], BF16, tag="osb")
                    nc.vector.scalar_tensor_tensor(o11[:], pst[:], 1.0, c2[:], MULT, ADD)
                    l3_out("c11", r, o11)

            es_evA.__exit__(None, None, None)
            es_osb.__exit__(None, None, None)
            es_wo.__exit__(None, None, None)
            es_d.__exit__(None, None, None)
            es_h2c21.__exit__(None, None, None)
            es_h2c12.__exit__(None, None, None)
            es_h2c22.__exit__(None, None, None)
            es_h2c11.__exit__(None, None, None)

    nc.compile()
    return nc


def _expand_mask(mask, t=TILE):
    return np.repeat(np.repeat(np.asarray(mask, dtype=bool), t, axis=0), t, axis=1)


def _pack_lhsT(w):
    """[d_m, d_k] -> [d_m/P, P, d_k] lhsT panels (partition = contraction)."""
    d_m, d_k = w.shape
    mt, kt = d_m // P, d_k // P
    return np.ascontiguousarray(
        w.reshape(mt, P, kt, P).transpose(0, 3, 2, 1).reshape(mt, P, d_k)
    )


def _strassen_a(w):
    m, k = w.shape
    mh, kh = m // 2, k // 2
    A11, A12 = w[:mh, :kh], w[:mh, kh:]
    A21, A22 = w[mh:, :kh], w[mh:, kh:]
    return {
        1: A11 + A22, 2: A21 + A22, 3: A11, 4: A22,
        5: A11 + A12, 6: A21 - A11, 7: A12 - A22,
    }


def _pack_bias(b):
    n = b.shape[0] // P
    return np.ascontiguousarray(b.reshape(n, P).T)


def _run(x, w1e, b1, w2e, b2, wo, bo, d_in, d_h, d_out, n_cores=N_CORES, trace=False):
    b = x.shape[0]
    bc = b // n_cores
    assert bc == 2 * NH

    nc = bacc.Bacc("TRN2", target_bir_lowering=False, debug=False, num_devices=n_cores)
    _build(nc, d_in, d_h, d_out, bc)

    np_bf16 = mybir.dt.np(BF16)

    def cvt(a):
        return np.ascontiguousarray(a.astype(np_bf16))

    shared = {}
    for name, w in (("w1", w1e), ("w2", w2e), ("wo", wo)):
        for j, a in _strassen_a(w).items():
            shared[f"{name}_{j}"] = cvt(_pack_lhsT(a))
    shared["b1t"] = _pack_bias(b1[:d_h // 2])
    shared["b1b"] = _pack_bias(b1[d_h // 2:])
    shared["b2t"] = _pack_bias(b2[:d_h // 2])
    shared["b2b"] = _pack_bias(b2[d_h // 2:])
    shared["bot"] = _pack_bias(bo[:d_out // 2])
    shared["bob"] = _pack_bias(bo[d_out // 2:])

    kh1 = d_in // 2
    kt1 = kh1 // P
    in_maps = []
    for c in range(n_cores):
        xc_ = np.ascontiguousarray(x[c * bc:(c + 1) * bc].T)  # [d_in, bc]
        B11 = xc_[:kh1, :NH]
        B12 = xc_[:kh1, NH:]
        B21 = xc_[kh1:, :NH]
        B22 = xc_[kh1:, NH:]
        m = {
            "xb11": cvt(B11.reshape(kt1, P, NH)),
            "xb22": cvt(
                B22.reshape(2, 4, P, NH).transpose(0, 2, 1, 3).reshape(2, P, 4 * NH)
            ),
        }
        for j, comb in (
            (1, B11 + B22), (3, B12 - B22), (4, B21 - B11),
            (6, B11 + B12), (7, B21 + B22),
        ):
            m[f"xc{j}"] = cvt(
                comb.reshape(kt1, P, NH).transpose(1, 0, 2).reshape(P, kt1 * NH)
            )
        in_maps.append({**m, **shared})

    res = run_bass_kernel_spmd(nc, in_maps, core_ids=list(range(n_cores)), trace=trace)
    outs = []
    rt3 = d_out // 2 // P
    for c in range(n_cores):
        o = res.results[c]["out"].reshape(4, rt3 * P, NH).astype(np.float32)
        full = np.empty((d_out, bc), np.float32)
        full[:rt3 * P, :NH] = o[0]
        full[:rt3 * P, NH:] = o[1]
        full[rt3 * P:, :NH] = o[2]
        full[rt3 * P:, NH:] = o[3]
        outs.append(full)
    full = np.concatenate(outs, axis=1)  # [d_out, B]
    return np.ascontiguousarray(full.T), res


def kernel(x, W1, b1, W2, b2, Wo, bo, mask1, mask2):
    x = np.asarray(x, dtype=np.float32)
    w1e = np.asarray(W1, dtype=np.float32) * _expand_mask(mask1)
    w2e = np.asarray(W2, dtype=np.float32) * _expand_mask(mask2)
    out, _ = _run(
        x,
        w1e,
        np.asarray(b1, np.float32),
        w2e,
        np.asarray(b2, np.float32),
        np.asarray(Wo, np.float32),
        np.asarray(bo, np.float32),
        d_in=2048,
        d_h=4096,
        d_out=1024,
    )
    return out
